# revision 44
# baseline (speedup 1.0000x reference)
"""Trainium2 Bass kernel for nn_AlignmentLayer (Kabsch alignment of L frames).

Strategy (pure data parallel over 8 NeuronCores, L/8 = 8192 frames per core):

Host-side (numpy, cheap layout work only):
  - ref_c = ref_x - mean(ref_x); gather xg = x[:, align_idx, :]  (align_idx is
    a host-known constant input, so the gather folds into data layout).
  - xgt: gathered atoms pre-transposed to [192, L] f32 so phase 1 needs
    zero on-chip transposes.
  - xsep: x in component-major layout [L, 3, 256] bf16 so phase-3 tensor ops
    are contiguous; output produced component-major bf16 and unpacked on host.
  - W: [192, 12] f32 weights mapping gathered rows to the 9 entries of
    A = xg^T @ ref_c and the 3 entries of the centroid x_c.

Device (per core), three phases:
  1. PE matmuls, weight-stationary: ET[12, ls] = W^T @ xgt in 512-frame
     strips (f32 for exact E — bf16 E perturbs near-singular frames), then
     PE-transposed back to E[128, nt*12] via identity matmuls.
  2. Math (DVE + Pool + ACT, batched [128, 64] ops): SVD-free Kabsch
     rotation. S = A^T A; lambda1 via trigonometric cubic (arctan+sin);
     v1 = best cross product of rows of (S - lambda1 I); (v2, v3) from a
     deflated 2x2 eigenproblem in the Householder complement of v1;
     u_i = normalize(A v_i); u3 = u1 x u2; R = sum u_i v_i^T; tneg = -x_c R.
     rsqrt/recip computed as Exp(-c*Ln(x)) on ACT; a greedy list scheduler
     splits the op DAG across DVE and Pool.
  3. Apply (bf16): per 128-frame tile and component b, products
     P0 = x0*R0b + tneg_b (ACT), P1 = x1*R1b, P2 = x2*R2b (DVE ts), then
     two 768-wide DVE adds produce the output tile.
"""

import numpy as np

L_FULL = 65536
N_INP = 256
N_ALIGN = 64
N_CORES = 8
LS = L_FULL // N_CORES          # frames per core
NT = LS // 128                  # 128-frame tiles per core (64)
F32 = np.float32

_RUNNER = None


# ----------------------------------------------------------------------------
# Math IR: record ops on virtual registers; a greedy list scheduler assigns
# each op to DVE ("V") or Pool ("G") (ACT ops pinned to "S"), then emission
# uses per-engine linear-scan slot allocation into one scratch tensor.
# ----------------------------------------------------------------------------

class _VR(int):
    """Virtual register id."""


# measured per-op engine costs at [128, 64] f32 (ns)
_COST = {
    ("tt", "V"): 150, ("tt", "G"): 300,
    ("ts", "V"): 115, ("ts", "G"): 260,
}
_ACT_COST = {"Ln": 240, "Exp": 350}
_XENG_NS = 250        # cross-engine result handoff penalty
_V_BIAS = 1.0         # apply follows math serially, so just balance math wall
_REBAL_NS = 800       # affinity hysteresis: rebalance only past this drift


class _MathIR:
    def __init__(self, alu):
        self.A_ = alu
        self.ops = []           # (kind, out, ins, extra)
        self.n = 0

    def _rec(self, kind, ins, extra=None, out=None):
        if out is None:
            out = _VR(self.n)
            self.n += 1
        self.ops.append((kind, out, list(ins), extra))
        return out

    def tt(self, op, a, b, out=None):
        return self._rec("tt", [a, b], op, out)

    def mul(self, a, b, out=None):
        return self.tt(self.A_.mult, a, b, out)

    def add(self, a, b, out=None):
        return self.tt(self.A_.add, a, b, out)

    def sub(self, a, b, out=None):
        return self.tt(self.A_.subtract, a, b, out)

    def ts(self, a, s1, op0, s2=None, op1=None, out=None):
        return self._rec("ts", [a], (float(s1), op0,
                                     None if s2 is None else float(s2), op1), out)

    def act(self, fn, a, scale=1.0, bias=None, out=None):
        return self._rec("act", [a], (fn, scale, bias), out)

    def rsqrt(self, nval):
        """1/sqrt(n) = Exp(-0.5*Ln(n)) on ACT (n must be > 0)."""
        from concourse import mybir
        AF = mybir.ActivationFunctionType
        ln = self.act(AF.Ln, nval)
        return self.act(AF.Exp, ln, scale=-0.5)

    def recip(self, nval):
        """1/n = Exp(-Ln(n)) on ACT (n must be > 0)."""
        from concourse import mybir
        AF = mybir.ActivationFunctionType
        ln = self.act(AF.Ln, nval)
        return self.act(AF.Exp, ln, scale=-1.0)

    def dot3(self, ax, ay, az, bx, by, bz):
        t1 = self.mul(ax, bx)
        t2 = self.mul(ay, by)
        s = self.add(t1, t2)
        t3 = self.mul(az, bz)
        return self.add(s, t3)

    def cross3(self, a, b):
        cx = self.sub(self.mul(a[1], b[2]), self.mul(a[2], b[1]))
        cy = self.sub(self.mul(a[2], b[0]), self.mul(a[0], b[2]))
        cz = self.sub(self.mul(a[0], b[1]), self.mul(a[1], b[0]))
        return [cx, cy, cz]

    def blend3(self, m, a, b):
        out = []
        for i in range(3):
            d = self.sub(a[i], b[i])
            out.append(self.add(b[i], self.mul(m, d)))
        return out


_RAW_LAT = 100        # same-engine RAW result latency (SBUF write ack)


def _schedule_math(ir):
    """Latency-aware list scheduling: all tensor ops on V, acts on S, and the
    EMISSION ORDER is chosen so dependent ops are spaced apart (back-to-back
    RAW chains pay the DVE write-ack latency). Returns (order, assign, clock).
    """
    n = len(ir.ops)
    eng, cost = [], []
    for kind, out, ins, extra in ir.ops:
        if kind == "act":
            fname = getattr(extra[0], "name", str(extra[0]))
            eng.append("S")
            cost.append(_ACT_COST.get(fname, 440))
        else:
            eng.append("V")
            cost.append(_COST[(kind, "V")])

    # dependency edges via vregs
    producer = {}
    deps = [[] for _ in range(n)]
    users = [[] for _ in range(n)]
    for i, (kind, out, ins, extra) in enumerate(ir.ops):
        for v in ins:
            if isinstance(v, _VR) and int(v) in producer:
                p = producer[int(v)]
                deps[i].append(p)
                users[p].append(i)
        if isinstance(out, _VR):
            producer[int(out)] = i

    # height = critical-path length to any sink
    height = [0] * n
    for i in range(n - 1, -1, -1):
        h = cost[i]
        for u in users[i]:
            h = max(h, cost[i] + height[u])
        height[i] = h

    indeg = [len(set(deps[i])) for i in range(n)]
    ready = [i for i in range(n) if indeg[i] == 0]
    clock = {"V": 0.0, "S": 0.0, "G": 0.0}
    fin = [0.0] * n
    done_deps = [set() for _ in range(n)]
    order = []
    import heapq
    while ready:
        # earliest feasible start per candidate
        best, best_key = None, None
        for i in ready:
            e = eng[i]
            est = clock[e]
            for p in set(deps[i]):
                lat = _RAW_LAT if eng[p] == e else _XENG_NS
                est = max(est, fin[p] + lat)
            stall = est - clock[e]
            key = (stall, -height[i])
            if best_key is None or key < best_key:
                best, best_key, best_est = i, key, est
        i = best
        ready.remove(i)
        e = eng[i]
        fin[i] = best_est + cost[i]
        clock[e] = fin[i]
        order.append(i)
        for u in users[i]:
            done_deps[u].add(i)
            if len(done_deps[u]) == len(set(deps[u])) and u not in ready \
                    and u not in order:
                ready.append(u)
    assert len(order) == n
    return order, eng, clock


def _emit_math(nc, ir, ms_ap, C, n_slots):
    """Emit recorded IR in the latency-aware schedule order. Vreg v lives in
    ms_ap[:, slot*C:(slot+1)*C]; slots partitioned per engine so WAR reuse
    stays engine-local."""
    order, assign, clock = _schedule_math(ir)

    # last use position in the EMISSION order
    pos = {op_i: k for k, op_i in enumerate(order)}
    last_use = {}
    for i, (kind, out, ins, extra) in enumerate(ir.ops):
        for v in ins:
            if isinstance(v, _VR):
                last_use[int(v)] = max(last_use.get(int(v), -1), pos[i])

    # per-engine slot ranges sized from peak live-value demand (emission order)
    peak = {"V": 0, "G": 0, "S": 0}
    live = {"V": 0, "G": 0, "S": 0}
    ends = {}
    for k, op_i in enumerate(order):
        kind, out, ins, extra = ir.ops[op_i]
        e = assign[op_i]
        if isinstance(out, _VR):
            live[e] += 1
            peak[e] = max(peak[e], live[e])
            ends[int(out)] = e
        for vi in {int(v) for v in ins if isinstance(v, _VR)}:
            if last_use.get(vi) == k and vi in ends:
                live[ends[vi]] -= 1
    need = {e: peak[e] + 1 for e in peak}
    assert sum(need.values()) <= n_slots, f"need {need} > {n_slots} slots"
    ranges, lo = {}, 0
    for e in ("V", "G", "S"):
        ranges[e] = (lo, lo + need[e])
        lo += need[e]
    free = {e: list(range(r[1] - 1, r[0] - 1, -1)) for e, r in ranges.items()}
    slot_of = {}
    eng_of_slot = {}

    def ap_of(v):
        if isinstance(v, _VR):
            s = slot_of[int(v)]
            return ms_ap[:, s * C:(s + 1) * C]
        return v  # external AP

    for k, op_i in enumerate(order):
        kind, out, ins, extra = ir.ops[op_i]
        e = assign[op_i]
        if isinstance(out, _VR):
            assert free[e], f"scratch slots exhausted for engine {e}"
            slot = free[e].pop()
            slot_of[int(out)] = slot
            eng_of_slot[slot] = e
            out_ap = ms_ap[:, slot * C:(slot + 1) * C]
        else:
            out_ap = out
        in_aps = [ap_of(v) for v in ins]
        eng = {"V": nc.vector, "G": nc.gpsimd, "S": nc.scalar}[e]
        if kind == "tt":
            eng.tensor_tensor(out_ap, in_aps[0], in_aps[1], extra)
        elif kind == "ts":
            s1, op0, s2, op1 = extra
            if s2 is None:
                eng.tensor_scalar(out_ap, in_aps[0], s1, None, op0)
            else:
                eng.tensor_scalar(out_ap, in_aps[0], s1, s2, op0, op1)
        elif kind == "act":
            fn, scale, bias = extra
            if bias is None:
                nc.scalar.activation(out_ap, in_aps[0], fn, scale=scale)
            else:
                nc.scalar.activation(out_ap, in_aps[0], fn, scale=scale,
                                     bias=bias)
        else:
            raise ValueError(kind)
        for vi in {int(v) for v in ins if isinstance(v, _VR)}:
            if last_use.get(vi) == k:
                s = slot_of[vi]
                free[eng_of_slot[s]].append(s)
    return clock


def _record_math(ir, Ev, Rv, consts):
    """Record the whole rotation math on the IR. Ev/Rv are [128, 12, C] views
    (strided entry slices); consts maps name -> [128,1] const AP."""
    from concourse import mybir
    AF = mybir.ActivationFunctionType
    A_ = ir.A_

    Ae = [[Ev[:, 3 * a + b, :] for b in range(3)] for a in range(3)]
    me = [Ev[:, 9 + a, :] for a in range(3)]

    # S = A^T A (6 unique entries)
    Smat = {}
    for bi in range(3):
        for ci in range(bi, 3):
            Smat[(bi, ci)] = ir.dot3(Ae[0][bi], Ae[1][bi], Ae[2][bi],
                                     Ae[0][ci], Ae[1][ci], Ae[2][ci])

    def S(i, j):
        return Smat[(min(i, j), max(i, j))]

    q = ir.ts(ir.add(ir.add(S(0, 0), S(1, 1)), S(2, 2)), 1.0 / 3.0, A_.mult)
    P00 = ir.sub(S(0, 0), q)
    P11 = ir.sub(S(1, 1), q)
    P22 = ir.sub(S(2, 2), q)
    sq01 = ir.mul(S(0, 1), S(0, 1))
    sq02 = ir.mul(S(0, 2), S(0, 2))
    sq12 = ir.mul(S(1, 2), S(1, 2))
    diagsq = ir.add(ir.add(ir.mul(P00, P00), ir.mul(P11, P11)), ir.mul(P22, P22))
    offsq = ir.add(ir.add(sq01, sq02), sq12)
    p2v = ir.add(diagsq, ir.ts(offsq, 2.0, A_.mult))
    p2c = ir.ts(ir.ts(p2v, 1.0 / 6.0, A_.mult), 1e-30, A_.max)
    ln_p = ir.act(AF.Ln, p2c)
    pval = ir.act(AF.Exp, ln_p, scale=0.5)       # sqrt(p2c)
    pinv3 = ir.act(AF.Exp, ln_p, scale=-1.5)     # p2c^-1.5

    c0 = ir.sub(ir.mul(P11, P22), sq12)
    c1c = ir.sub(ir.mul(S(0, 1), P22), ir.mul(S(1, 2), S(0, 2)))
    c2c = ir.sub(ir.mul(S(0, 1), S(1, 2)), ir.mul(P11, S(0, 2)))
    detB = ir.add(ir.sub(ir.mul(P00, c0), ir.mul(S(0, 1), c1c)),
                  ir.mul(S(0, 2), c2c))
    rr = ir.ts(ir.mul(detB, pinv3), 0.5, A_.mult, 0.9999995, A_.min)
    rr = ir.ts(rr, -0.9999995, A_.max)

    omr = ir.ts(ir.mul(rr, rr), -1.0, A_.mult, 1.0, A_.add)
    rs = ir.rsqrt(omr)
    uu = ir.mul(rr, rs)
    # arctan(u) with range reduction — ACT Arctan domain is [-pi/2, pi/2]:
    # |u|<=1: a = arctan(|u|); |u|>1: pi/2 - arctan(1/|u|); then apply sign.
    au = ir.tt(A_.max, uu, ir.ts(uu, -1.0, A_.mult))      # |u|
    inv = ir.recip(ir.ts(au, 1e-30, A_.max))
    z = ir.tt(A_.min, au, inv)
    az = ir.act(AF.Arctan, z)
    dz = ir.ts(az, -1.0, A_.mult, float(np.pi / 2), A_.add)
    mge = ir.ts(au, 1.0, A_.is_ge)                        # |u| >= 1
    mle = ir.act(AF.Identity, mge, scale=-1.0, bias=consts["one"])  # 1 - that
    res_abs = ir.add(dz, ir.mul(mle, ir.sub(az, dz)))
    sgn_u = ir.ts(ir.ts(uu, 0.0, A_.is_ge), 2.0, A_.mult, -1.0, A_.add)
    at = ir.mul(res_abs, sgn_u)
    c1t = ir.act(AF.Sin, at, scale=1.0 / 3.0, bias=consts["pi3"])
    lam1 = ir.add(q, ir.ts(ir.mul(pval, c1t), 2.0, A_.mult))

    # v1 = best cross of rows of (S - lam1 I)
    D0 = ir.sub(S(0, 0), lam1)
    D1 = ir.sub(S(1, 1), lam1)
    D2 = ir.sub(S(2, 2), lam1)
    rows = [
        [D0, S(0, 1), S(0, 2)],
        [S(0, 1), D1, S(1, 2)],
        [S(0, 2), S(1, 2), D2],
    ]
    best, bn = None, None
    for (i, j) in [(0, 1), (0, 2)]:
        c = ir.cross3(rows[i], rows[j])
        n = ir.dot3(c[0], c[1], c[2], c[0], c[1], c[2])
        if best is None:
            best, bn = c, n
        else:
            m = ir.tt(A_.is_gt, n, bn)
            best = ir.blend3(m, c, best)
            bn = ir.add(bn, ir.mul(m, ir.sub(n, bn)))
    inv1 = ir.rsqrt(ir.ts(bn, 1e-37, A_.max))
    v1 = [ir.mul(best[0], inv1), ir.mul(best[1], inv1), ir.mul(best[2], inv1)]

    # (w2, w3): orthonormal complement of v1 via Householder columns.
    # H = I - h h^T/(1+a), h = v1 + s*e0, s = sign(v1x), a = s*v1x = |v1x|.
    sgn = ir.ts(ir.ts(v1[0], 0.0, A_.is_ge), 2.0, A_.mult, -1.0, A_.add)
    alpha = ir.mul(sgn, v1[0])
    denom = ir.ts(alpha, 1.0, A_.add)                     # 1 + |v1x| in [1,2]
    rden = ir.recip(denom)
    h0 = ir.add(v1[0], sgn)
    hyr = ir.mul(v1[1], rden)
    nhyr = ir.ts(hyr, -1.0, A_.mult)
    w2 = [ir.mul(h0, nhyr),
          ir.ts(ir.mul(v1[1], hyr), -1.0, A_.mult, 1.0, A_.add),
          ir.mul(v1[2], nhyr)]
    w3 = ir.cross3(v1, w2)

    # deflated 2x2 eigenproblem in span{w2, w3}; c2x via trace identity.
    Sw2 = [ir.dot3(S(bi, 0), S(bi, 1), S(bi, 2), w2[0], w2[1], w2[2])
           for bi in range(3)]
    a2x = ir.dot3(w2[0], w2[1], w2[2], Sw2[0], Sw2[1], Sw2[2])
    b2x = ir.dot3(Sw2[0], Sw2[1], Sw2[2], w3[0], w3[1], w3[2])
    trq = ir.act(AF.Identity, q, scale=3.0)
    c2x = ir.sub(trq, ir.add(lam1, a2x))

    half = ir.ts(ir.sub(a2x, c2x), 0.5, A_.mult)
    mpos = ir.ts(half, 0.0, A_.is_ge)
    sgn2 = ir.ts(mpos, 2.0, A_.mult, -1.0, A_.add)
    habs = ir.mul(sgn2, half)
    rad2 = ir.ts(ir.add(ir.mul(half, half), ir.mul(b2x, b2x)), 1e-37, A_.max)
    rad = ir.act(AF.Sqrt, rad2)
    pos = ir.ts(ir.add(habs, rad), 1e-37, A_.max)
    tq = ir.mul(ir.mul(b2x, ir.recip(pos)), sgn2)
    c2i = ir.rsqrt(ir.ts(ir.mul(tq, tq), 1.0, A_.add))
    s2i = ir.mul(tq, c2i)
    tb = ir.mul(tq, b2x)
    lamA = ir.add(a2x, tb)
    lamB = ir.sub(c2x, tb)
    mAB = ir.tt(A_.is_ge, lamA, lamB)
    vA = [ir.add(ir.mul(c2i, w2[i]), ir.mul(s2i, w3[i])) for i in range(3)]
    vB = [ir.sub(ir.mul(c2i, w3[i]), ir.mul(s2i, w2[i])) for i in range(3)]
    v2 = ir.blend3(mAB, vA, vB)
    v3 = ir.cross3(v1, v2)

    def Avec(v):
        return [ir.dot3(Ae[ai][0], Ae[ai][1], Ae[ai][2], v[0], v[1], v[2])
                for ai in range(3)]

    b1 = Avec(v1)
    n1 = ir.dot3(b1[0], b1[1], b1[2], b1[0], b1[1], b1[2])
    i1 = ir.rsqrt(ir.ts(n1, 1e-37, A_.max))
    u1 = [ir.mul(b1[i], i1) for i in range(3)]

    b2v = Avec(v2)
    dd = ir.dot3(u1[0], u1[1], u1[2], b2v[0], b2v[1], b2v[2])
    b2o = [ir.sub(b2v[i], ir.mul(dd, u1[i])) for i in range(3)]
    n2 = ir.dot3(b2o[0], b2o[1], b2o[2], b2o[0], b2o[1], b2o[2])
    i2 = ir.rsqrt(ir.ts(n2, 1e-37, A_.max))
    u2 = [ir.mul(b2o[i], i2) for i in range(3)]

    u3 = ir.cross3(u1, u2)

    us = [u1, u2, u3]
    vs = [v1, v2, v3]
    Re = [[None] * 3 for _ in range(3)]
    for ai in range(3):
        for bi in range(3):
            t1 = ir.mul(us[0][ai], vs[0][bi])
            t2 = ir.mul(us[1][ai], vs[1][bi])
            sgm = ir.add(t1, t2)
            t3 = ir.mul(us[2][ai], vs[2][bi])
            r = ir.add(sgm, t3)
            Re[ai][bi] = r
            ir.act(AF.Identity, r, out=Rv[:, 3 * ai + bi, :])

    mn = [ir.act(AF.Identity, me[i], scale=-1.0) for i in range(3)]
    for bi in range(3):
        t1 = ir.mul(mn[0], Re[0][bi])
        t2 = ir.mul(mn[1], Re[1][bi])
        sgm = ir.add(t1, t2)
        t3 = ir.mul(mn[2], Re[2][bi])
        ir.add(sgm, t3, out=Rv[:, 9 + bi, :])


# ----------------------------------------------------------------------------
# Bass program
# ----------------------------------------------------------------------------

def _split_multiwait(nc):
    """This walrus build encodes at most ONE semaphore wait per instruction,
    but Tile emits several. Split extras into standalone EventSemaphore
    (pure wait) instructions on the same engine, immediately before."""
    from concourse import mybir
    import bass_rust

    n_split = 0
    for fn in nc.m.functions:
        for blk in fn.blocks:
            new = []
            for ins in blk.instructions:
                si = ins.sync_info
                if si is not None and si.on_wait is not None and len(si.on_wait) > 1:
                    waits = list(si.on_wait)
                    for k, w in enumerate(waits[:-1]):
                        new.append(mybir.InstEventSemaphore(
                            name=f"{ins.name}-w{k}",
                            engine=ins.engine,
                            sync_info=bass_rust.SyncInfo(
                                on_wait=[w], on_update=[]),
                        ))
                        n_split += 1
                    ins.sync_info = bass_rust.SyncInfo(
                        on_wait=[waits[-1]],
                        on_update=list(si.on_update or []))
                new.append(ins)
            blk.instructions = new
    return n_split


def _build_program(ls=LS, n_slots=62, split_waits=True, prefetch=6):
    import concourse.bass as bass
    import concourse.tile as tile
    from concourse import mybir

    f32 = mybir.dt.float32
    bf16 = mybir.dt.bfloat16
    A_ = mybir.AluOpType
    AF = mybir.ActivationFunctionType

    nt = ls // 128
    C = nt

    nc = bass.Bass("TRN2", target_bir_lowering=False, debug=False)

    f32r = mybir.dt.float32r
    xgt_d = nc.dram_tensor("xgt", [192, ls], f32r, kind="ExternalInput").ap()
    xsep_d = nc.dram_tensor("xsep", [ls, 768], bf16, kind="ExternalInput").ap()
    w_d = nc.dram_tensor("wm", [192, 12], f32r, kind="ExternalInput").ap()
    id_d = nc.dram_tensor("ident", [12, 12], f32, kind="ExternalInput").ap()
    idb_d = nc.dram_tensor("identb", [128, 128], bf16, kind="ExternalInput").ap()
    out_d = nc.dram_tensor("out", [ls, 768], bf16, kind="ExternalOutput").ap()

    with tile.TileContext(nc) as tc:
        with (
            tc.tile_pool(name="wp", bufs=1) as wp,
            tc.tile_pool(name="gp_", bufs=1) as gpool,
            tc.tile_pool(name="ep", bufs=1) as ep,
            tc.tile_pool(name="xp", bufs=prefetch) as xp,
            tc.tile_pool(name="p2", bufs=2) as p2p,
            tc.tile_pool(name="op_", bufs=3) as opool,
        ):
            # ---------------- constants / weights ----------------
            identb = wp.tile([128, 128], bf16, tag="identb")
            nc.sync.dma_start(identb[:], idb_d)
            w0 = wp.tile([128, 12], f32r, tag="w0")
            w1 = wp.tile([64, 12], f32r, tag="w1")
            nc.sync.dma_start(w0[:], w_d[0:128, :])
            nc.sync.dma_start(w1[:], w_d[128:192, :])
            ident = wp.tile([12, 12], f32, tag="ident")
            nc.sync.dma_start(ident[:], id_d)

            ET = ep.tile([12, ls], f32, tag="ET")
            # E and R are ENTRY-MAJOR [128, e*nt + g]: math operands become
            # contiguous 64-column slices (DVE fast path, Pool software loop).
            E = ep.tile([128, nt * 12], f32, tag="E")
            R = ep.tile([128, nt * 12], f32, tag="R")
            MS = ep.tile([128, n_slots * C], f32, tag="MS")
            cst = ep.tile([128, 4], f32, tag="cst")
            nc.gpsimd.memset(cst[:, 0:1], float(np.pi / 3))
            nc.gpsimd.memset(cst[:, 1:2], float(np.pi / 2))
            nc.gpsimd.memset(cst[:, 2:3], 1.0)
            nc.gpsimd.memset(cst[:, 3:4], -1.0)
            consts = {"pi3": cst[:, 0:1], "pi2": cst[:, 1:2],
                      "one": cst[:, 2:3], "neg1": cst[:, 3:4]}
            Ev = E[:].rearrange("p (e g) -> p e g", e=12)
            Rv = R[:].rearrange("p (e g) -> p e g", e=12)
            Eg = E[:].rearrange("p (e g) -> p g e", e=12)

            # ---------------- phase 1: ET = W^T @ xgt (f32r), then PE ------
            # transposes back to E. Dummy PE matmuls absorb each DMA's
            # semaphore into the PE's observed clock (Matmult ISA slot holds
            # at most ONE wait). Phase-1 PSUM pools are scoped so their banks
            # free up for the apply-phase accumulator pool.
            psp = tc.alloc_tile_pool(name="ps", bufs=2, space="PSUM")
            pstp = tc.alloc_tile_pool(name="pst", bufs=2, space="PSUM")
            pss = tc.alloc_tile_pool(name="ps2", bufs=1, space="PSUM")
            ps_scr = pss.tile([128, 12], f32, tag="scr")
            nc.tensor.matmul(ps_scr[0:12, 0:12], w0[:, 0:12], w0[:],
                             start=True, stop=True)
            nc.tensor.matmul(ps_scr[0:12, 0:12], w1[:, 0:12], w1[:],
                             start=True, stop=True)
            n_strip = ls // 512
            slabs = []
            for s in range(n_strip):
                sl0 = gpool.tile([128, 512], f32r, tag=f"g0_{s}")
                sl1 = gpool.tile([64, 512], f32r, tag=f"g1_{s}")
                eng_q = nc.sync if s % 2 == 0 else nc.scalar
                eng_q.dma_start(sl0[:], xgt_d[0:128, s * 512:(s + 1) * 512])
                eng_q.dma_start(sl1[:], xgt_d[128:192, s * 512:(s + 1) * 512])
                slabs.append((sl0, sl1))
            for s in range(n_strip):
                sl0, sl1 = slabs[s]
                nc.tensor.matmul(ps_scr[0:12, 0:12], sl0[:, 0:12], sl0[:, 0:12],
                                 start=True, stop=True)
                nc.tensor.matmul(ps_scr[0:12, 0:12], sl1[:, 0:12], sl1[:, 0:12],
                                 start=True, stop=True)
                psET = psp.tile([12, 512], f32, tag="psET")
                nc.tensor.matmul(psET[:], w0[:], sl0[:], start=True, stop=False)
                nc.tensor.matmul(psET[:], w1[:], sl1[:], start=False, stop=True)
                nc.scalar.copy(ET[:, s * 512:(s + 1) * 512], psET[:])
                if s % 2 == 1:
                    gb = s // 2      # transpose the 8 groups of strips s-1, s
                    psT = pstp.tile([128, 96], f32, tag="psT")
                    for k in range(8):
                        g = gb * 8 + k
                        nc.tensor.transpose(psT[:, k * 12:(k + 1) * 12],
                                            ET[:, g * 128:(g + 1) * 128],
                                            ident[:])
                    nc.scalar.copy(
                        Eg[:, gb * 8:(gb + 1) * 8, :],
                        psT[:].rearrange("p (g e) -> p g e", e=12))

            pss.release()
            pstp.release()
            psp.release()
            psop = tc.alloc_tile_pool(name="pso", bufs=2, space="PSUM")
            psc2 = tc.alloc_tile_pool(name="psc2", bufs=1, space="PSUM")
            scr2 = psc2.tile([16, 12], mybir.dt.float32, tag="scr2")

            # ---------------- phase 2: rotation math ----------------------
            ir = _MathIR(A_)
            _record_math(ir, Ev, Rv, consts)
            _emit_math(nc, ir, MS[:], C, n_slots)

            # ---------------- phase 3: apply (bf16) -----------------------
            # xq prefetch from the (idle) Pool queue, gated behind a junk
            # read of the last xgt strip so the prefetch transfers never
            # race the phase-1 strips on the DMA rings.
            junk = ep.tile([64, 16], f32, tag="junk")
            nc.gpsimd.tensor_scalar(junk[:], slabs[-1][1][:, 0:16].bitcast(f32),
                                    1.0, None, A_.mult)
            n_grp = nt // 4
            xqs = []
            for grp in range(n_grp):
                xq = xp.tile([128, 4 * 768], bf16, tag="xq")
                src = xsep_d[grp * 512:(grp + 1) * 512, :].rearrange(
                    "(g p) c -> p g c", p=128)
                nc.gpsimd.dma_start(xq[:].rearrange("p (g c) -> p g c", c=768), src)
                xqs.append(xq)
            # apply: DVE/ACT compute the 9 per-component products into
            # pair-sized bf16 tiles; the (otherwise idle) TensorEngine sums
            # P0+P1+P2 into PSUM via identity matmuls; Pool casts PSUM->bf16
            # SBUF; SP DMAs out. DVE never runs a wide add.
            for pr in range(nt // 2):
                P0 = p2p.tile([128, 1536], bf16, tag="P0")
                P1 = p2p.tile([128, 1536], bf16, tag="P1")
                P2 = p2p.tile([128, 1536], bf16, tag="P2")
                for half in range(2):
                    gg = pr * 2 + half
                    grp, t = gg // 4, gg % 4
                    xq = xqs[grp]
                    base = t * 768
                    hb = half * 768
                    for bi in range(3):
                        rcol0 = R[:, bi * nt + gg: bi * nt + gg + 1]
                        rcol1 = R[:, (3 + bi) * nt + gg: (3 + bi) * nt + gg + 1]
                        rcol2 = R[:, (6 + bi) * nt + gg: (6 + bi) * nt + gg + 1]
                        tncol = R[:, (9 + bi) * nt + gg: (9 + bi) * nt + gg + 1]
                        x0 = xq[:, base:base + 256]
                        x1 = xq[:, base + 256:base + 512]
                        x2 = xq[:, base + 512:base + 768]
                        p0s = P0[:, hb + bi * 256:hb + (bi + 1) * 256]
                        nc.scalar.activation(p0s, x0, AF.Identity,
                                             bias=tncol, scale=rcol0)
                        nc.vector.tensor_scalar(
                            P1[:, hb + bi * 256:hb + (bi + 1) * 256],
                            x1, rcol1, None, A_.mult)
                        nc.vector.tensor_scalar(
                            P2[:, hb + bi * 256:hb + (bi + 1) * 256],
                            x2, rcol2, None, A_.mult)
                # PE sums P0+P1+P2 into PSUM. The Matmult ISA slot holds at
                # most ONE semaphore wait, so three dummy matmuls absorb the
                # ACT-sem, DVE-sem and psO-WAR-sem into the PE's observed
                # clock first; the real matmuls then need no fresh waits.
                psO = psop.tile([128, 1536], f32, tag="psO")
                nc.tensor.matmul(scr2[:], P0[:, 1520:1536], P0[:, 1520:1532],
                                 start=True, stop=True, skip_group_check=True)
                nc.tensor.matmul(scr2[:], P2[:, 1520:1536], P2[:, 1520:1532],
                                 start=True, stop=True, skip_group_check=True)
                nc.tensor.matmul(psO[0:16, 0:16], identb[:, 0:16],
                                 identb[:, 0:16], start=True, stop=True,
                                 skip_group_check=True)
                for c0 in (0, 512, 1024):
                    nc.tensor.matmul(psO[:, c0:c0 + 512], identb[:],
                                     P0[:, c0:c0 + 512], start=True, stop=False)
                    nc.tensor.matmul(psO[:, c0:c0 + 512], identb[:],
                                     P1[:, c0:c0 + 512], start=False, stop=False)
                    nc.tensor.matmul(psO[:, c0:c0 + 512], identb[:],
                                     P2[:, c0:c0 + 512], start=False, stop=True)
                ot = opool.tile([128, 1536], bf16, tag="ot")
                for ci, c0 in enumerate((0, 512, 1024)):
                    on_v = ci == 0 or (ci == 1 and pr % 5 != 4)
                    if on_v:
                        nc.vector.tensor_scalar(ot[:, c0:c0 + 512],
                                                psO[:, c0:c0 + 512],
                                                1.0, None, A_.mult)
                    else:
                        nc.scalar.activation(ot[:, c0:c0 + 512],
                                             psO[:, c0:c0 + 512], AF.Identity)
                gg0 = pr * 2
                dst = out_d[gg0 * 128:(gg0 + 2) * 128, :].rearrange(
                    "(g p) c -> p g c", p=128)
                nc.sync.dma_start(dst, ot[:].rearrange(
                    "p (g c) -> p g c", c=768))
            psc2.release()
            psop.release()

    if split_waits:
        _split_multiwait(nc)
    return nc


# ----------------------------------------------------------------------------
# Host-side preparation
# ----------------------------------------------------------------------------

def _prep_inputs(x, ref_x, align_idx):
    import ml_dtypes
    BF16 = ml_dtypes.bfloat16
    x = np.asarray(x, dtype=F32)
    ref_x = np.asarray(ref_x)
    idx = np.asarray(align_idx).astype(np.int64)
    L = x.shape[0]

    ref64 = ref_x.astype(np.float64)
    ref_c = (ref64 - ref64.mean(0)).astype(F32)        # [64, 3]

    xg = x[:, idx, :]                                   # [L, 64, 3]
    xgt = np.ascontiguousarray(xg.reshape(L, 192).T)    # f32 [192, L]

    xsep = np.ascontiguousarray(
        x.transpose(0, 2, 1)).reshape(L, 768).astype(BF16)

    W = np.zeros((192, 12), dtype=F32)
    for a in range(3):
        rows = 3 * np.arange(N_ALIGN) + a
        for b in range(3):
            W[rows, 3 * a + b] = ref_c[:, b]
        W[rows, 9 + a] = F32(1.0 / N_ALIGN)
    return xgt, xsep, W


# ----------------------------------------------------------------------------
# Runner: jit once, reuse
# ----------------------------------------------------------------------------

class _Runner:
    def __init__(self):
        import jax

        self.jax = jax
        self.nc = _build_program(LS)
        self._build_exec()

    def _build_exec(self):
        import jax
        from jax.sharding import Mesh, PartitionSpec
        from jax.experimental.shard_map import shard_map
        from concourse import mybir
        from concourse.bass2jax import (_bass_exec_p, install_neuronx_cc_hook,
                                        partition_id_tensor)

        install_neuronx_cc_hook()
        # surface compile-hook exceptions (PJRT swallows them)
        try:
            import libneuronxla
            import traceback
            if not getattr(libneuronxla, "_ant_logged_cc", False):
                _orig_cc = libneuronxla.neuronx_cc

                def _logged_cc(*a, **k):
                    try:
                        return _orig_cc(*a, **k)
                    except BaseException:
                        traceback.print_exc()
                        raise

                libneuronxla.neuronx_cc = _logged_cc
                libneuronxla._ant_logged_cc = True
        except ImportError:
            pass
        nc = self.nc

        part_name = (nc.partition_id_tensor.name
                     if nc.partition_id_tensor else None)
        in_names, out_names, out_avals = [], [], []
        for alloc in nc.m.functions[0].allocations:
            if not isinstance(alloc, mybir.MemoryLocationSet):
                continue
            name = alloc.memorylocations[0].name
            if alloc.kind == "ExternalInput":
                if name != part_name:
                    in_names.append(name)
            elif alloc.kind == "ExternalOutput":
                shape = tuple(alloc.tensor_shape)
                dtype = mybir.dt.np(alloc.dtype)
                out_names.append(name)
                out_avals.append(jax.core.ShapedArray(shape, dtype))
        self.in_names = list(in_names)
        self.out_names = list(out_names)
        n_params = len(in_names)
        all_names = in_names + out_names
        if part_name is not None:
            all_names = all_names + [part_name]

        def _body(*args):
            operands = list(args)
            if part_name is not None:
                operands.append(partition_id_tensor())
            outs = _bass_exec_p.bind(
                *operands,
                out_avals=tuple(out_avals),
                in_names=tuple(all_names),
                out_names=tuple(out_names),
                lowering_input_output_aliases=(),
                sim_require_finite=True,
                sim_require_nnan=True,
                nc=nc,
            )
            return tuple(outs)

        devices = jax.devices()[:N_CORES]
        mesh = Mesh(np.asarray(devices), ("core",))
        n_outs = len(out_names)
        in_specs = (PartitionSpec("core"),) * (n_params + n_outs)
        out_specs = (PartitionSpec("core"),) * n_outs
        self._fn = jax.jit(
            shard_map(_body, mesh=mesh, in_specs=in_specs,
                      out_specs=out_specs, check_rep=False),
            keep_unused=True,
        )
        self._zeros = [
            np.zeros((N_CORES * av.shape[0], *av.shape[1:]), av.dtype)
            for av in out_avals
        ]

    def stage(self, x, ref_x, align_idx):
        import ml_dtypes
        xgt, xsep, W = _prep_inputs(x, ref_x, align_idx)
        per_name = {
            "xgt": np.concatenate(
                [xgt[:, c * LS:(c + 1) * LS] for c in range(N_CORES)], axis=0),
            "xsep": xsep,
            "wm": np.concatenate([W] * N_CORES, axis=0),
            "ident": np.concatenate(
                [np.eye(12, dtype=F32)] * N_CORES, axis=0),
            "identb": np.concatenate(
                [np.eye(128).astype(ml_dtypes.bfloat16)] * N_CORES, axis=0),
        }
        args = [per_name[n] for n in self.in_names] + list(self._zeros)
        return [self.jax.device_put(a) for a in args]

    def run_staged(self, staged):
        return self._fn(*staged)

    def run(self, x, ref_x, align_idx):
        staged = self.stage(x, ref_x, align_idx)
        outs = self.run_staged(staged)
        out = np.asarray(outs[self.out_names.index("out")]).astype(np.float32)
        L = out.shape[0]
        return np.ascontiguousarray(
            out.reshape(L, 3, N_INP).transpose(0, 2, 1))


def _get_runner():
    global _RUNNER
    if _RUNNER is None:
        _RUNNER = _Runner()
    return _RUNNER


def kernel(x, ref_x, align_idx):
    runner = _get_runner()
    return runner.run(x, ref_x, align_idx).astype(np.float32)


if __name__ == "__main__":
    nc = _build_program(LS)
    print("built ok")


# revision 45
# speedup vs baseline: 1.0881x; 1.0881x over previous
"""Trainium2 Bass kernel for nn_AlignmentLayer (Kabsch alignment of L frames).

Strategy (pure data parallel over 8 NeuronCores, L/8 = 8192 frames per core):

Host-side (numpy, cheap layout work only):
  - ref_c = ref_x - mean(ref_x); gather xg = x[:, align_idx, :]  (align_idx is
    a host-known constant input, so the gather folds into data layout).
  - xgt: gathered atoms pre-transposed to [192, L] f32 so phase 1 needs
    zero on-chip transposes.
  - xsep: x in component-major layout [L, 3, 256] bf16 so phase-3 tensor ops
    are contiguous; output produced component-major bf16 and unpacked on host.
  - W: [192, 12] f32 weights mapping gathered rows to the 9 entries of
    A = xg^T @ ref_c and the 3 entries of the centroid x_c.

Device (per core), three phases:
  1. PE matmuls, weight-stationary: ET[12, ls] = W^T @ xgt in 512-frame
     strips (f32 for exact E — bf16 E perturbs near-singular frames), then
     PE-transposed back to E[128, nt*12] via identity matmuls.
  2. Math (DVE + Pool + ACT, batched [128, 64] ops): SVD-free Kabsch
     rotation. S = A^T A; lambda1 via trigonometric cubic (arctan+sin);
     v1 = best cross product of rows of (S - lambda1 I); (v2, v3) from a
     deflated 2x2 eigenproblem in the Householder complement of v1;
     u_i = normalize(A v_i); u3 = u1 x u2; R = sum u_i v_i^T; tneg = -x_c R.
     rsqrt/recip computed as Exp(-c*Ln(x)) on ACT; a greedy list scheduler
     splits the op DAG across DVE and Pool.
  3. Apply (bf16): per 128-frame tile and component b, products
     P0 = x0*R0b + tneg_b (ACT), P1 = x1*R1b, P2 = x2*R2b (DVE ts), then
     two 768-wide DVE adds produce the output tile.
"""

import numpy as np

L_FULL = 65536
N_INP = 256
N_ALIGN = 64
N_CORES = 8
LS = L_FULL // N_CORES          # frames per core
NT = LS // 128                  # 128-frame tiles per core (64)
F32 = np.float32

_RUNNER = None


# ----------------------------------------------------------------------------
# Math IR: record ops on virtual registers; a greedy list scheduler assigns
# each op to DVE ("V") or Pool ("G") (ACT ops pinned to "S"), then emission
# uses per-engine linear-scan slot allocation into one scratch tensor.
# ----------------------------------------------------------------------------

class _VR(int):
    """Virtual register id."""


# measured per-op engine costs at [128, 64] f32 (ns)
_COST = {
    ("tt", "V"): 150, ("tt", "G"): 300,
    ("ts", "V"): 115, ("ts", "G"): 260,
}
_ACT_COST = {"Ln": 240, "Exp": 350}
_XENG_NS = 250        # cross-engine result handoff penalty
_V_BIAS = 1.0         # apply follows math serially, so just balance math wall
_REBAL_NS = 800       # affinity hysteresis: rebalance only past this drift


class _MathIR:
    def __init__(self, alu):
        self.A_ = alu
        self.ops = []           # (kind, out, ins, extra)
        self.n = 0

    def _rec(self, kind, ins, extra=None, out=None):
        if out is None:
            out = _VR(self.n)
            self.n += 1
        self.ops.append((kind, out, list(ins), extra))
        return out

    def tt(self, op, a, b, out=None):
        return self._rec("tt", [a, b], op, out)

    def mul(self, a, b, out=None):
        return self.tt(self.A_.mult, a, b, out)

    def add(self, a, b, out=None):
        return self.tt(self.A_.add, a, b, out)

    def sub(self, a, b, out=None):
        return self.tt(self.A_.subtract, a, b, out)

    def ts(self, a, s1, op0, s2=None, op1=None, out=None):
        return self._rec("ts", [a], (float(s1), op0,
                                     None if s2 is None else float(s2), op1), out)

    def act(self, fn, a, scale=1.0, bias=None, out=None):
        return self._rec("act", [a], (fn, scale, bias), out)

    def rsqrt(self, nval):
        """1/sqrt(n) = Exp(-0.5*Ln(n)) on ACT (n must be > 0)."""
        from concourse import mybir
        AF = mybir.ActivationFunctionType
        ln = self.act(AF.Ln, nval)
        return self.act(AF.Exp, ln, scale=-0.5)

    def recip(self, nval):
        """1/n = Exp(-Ln(n)) on ACT (n must be > 0)."""
        from concourse import mybir
        AF = mybir.ActivationFunctionType
        ln = self.act(AF.Ln, nval)
        return self.act(AF.Exp, ln, scale=-1.0)

    def dot3(self, ax, ay, az, bx, by, bz):
        t1 = self.mul(ax, bx)
        t2 = self.mul(ay, by)
        s = self.add(t1, t2)
        t3 = self.mul(az, bz)
        return self.add(s, t3)

    def cross3(self, a, b):
        cx = self.sub(self.mul(a[1], b[2]), self.mul(a[2], b[1]))
        cy = self.sub(self.mul(a[2], b[0]), self.mul(a[0], b[2]))
        cz = self.sub(self.mul(a[0], b[1]), self.mul(a[1], b[0]))
        return [cx, cy, cz]

    def blend3(self, m, a, b):
        out = []
        for i in range(3):
            d = self.sub(a[i], b[i])
            out.append(self.add(b[i], self.mul(m, d)))
        return out


_RAW_LAT = 100        # same-engine RAW result latency (SBUF write ack)


def _schedule_math(ir):
    """Latency-aware list scheduling: all tensor ops on V, acts on S, and the
    EMISSION ORDER is chosen so dependent ops are spaced apart (back-to-back
    RAW chains pay the DVE write-ack latency). Returns (order, assign, clock).
    """
    n = len(ir.ops)
    eng, cost = [], []
    for kind, out, ins, extra in ir.ops:
        if kind == "act":
            fname = getattr(extra[0], "name", str(extra[0]))
            eng.append("S")
            cost.append(_ACT_COST.get(fname, 440))
        else:
            eng.append("V")
            cost.append(_COST[(kind, "V")])

    # dependency edges via vregs
    producer = {}
    deps = [[] for _ in range(n)]
    users = [[] for _ in range(n)]
    for i, (kind, out, ins, extra) in enumerate(ir.ops):
        for v in ins:
            if isinstance(v, _VR) and int(v) in producer:
                p = producer[int(v)]
                deps[i].append(p)
                users[p].append(i)
        if isinstance(out, _VR):
            producer[int(out)] = i

    # height = critical-path length to any sink
    height = [0] * n
    for i in range(n - 1, -1, -1):
        h = cost[i]
        for u in users[i]:
            h = max(h, cost[i] + height[u])
        height[i] = h

    indeg = [len(set(deps[i])) for i in range(n)]
    ready = [i for i in range(n) if indeg[i] == 0]
    clock = {"V": 0.0, "S": 0.0, "G": 0.0}
    fin = [0.0] * n
    done_deps = [set() for _ in range(n)]
    order = []
    import heapq
    while ready:
        # earliest feasible start per candidate
        best, best_key = None, None
        for i in ready:
            e = eng[i]
            est = clock[e]
            for p in set(deps[i]):
                lat = _RAW_LAT if eng[p] == e else _XENG_NS
                est = max(est, fin[p] + lat)
            stall = est - clock[e]
            key = (stall, -height[i])
            if best_key is None or key < best_key:
                best, best_key, best_est = i, key, est
        i = best
        ready.remove(i)
        e = eng[i]
        fin[i] = best_est + cost[i]
        clock[e] = fin[i]
        order.append(i)
        for u in users[i]:
            done_deps[u].add(i)
            if len(done_deps[u]) == len(set(deps[u])) and u not in ready \
                    and u not in order:
                ready.append(u)
    assert len(order) == n
    return order, eng, clock


def _emit_math(nc, ir, ms_ap, C, n_slots):
    """Emit recorded IR in the latency-aware schedule order. Vreg v lives in
    ms_ap[:, slot*C:(slot+1)*C]; slots partitioned per engine so WAR reuse
    stays engine-local."""
    order, assign, clock = _schedule_math(ir)

    # last use position in the EMISSION order
    pos = {op_i: k for k, op_i in enumerate(order)}
    last_use = {}
    for i, (kind, out, ins, extra) in enumerate(ir.ops):
        for v in ins:
            if isinstance(v, _VR):
                last_use[int(v)] = max(last_use.get(int(v), -1), pos[i])

    # per-engine slot ranges sized from peak live-value demand (emission order)
    peak = {"V": 0, "G": 0, "S": 0}
    live = {"V": 0, "G": 0, "S": 0}
    ends = {}
    for k, op_i in enumerate(order):
        kind, out, ins, extra = ir.ops[op_i]
        e = assign[op_i]
        if isinstance(out, _VR):
            live[e] += 1
            peak[e] = max(peak[e], live[e])
            ends[int(out)] = e
        for vi in {int(v) for v in ins if isinstance(v, _VR)}:
            if last_use.get(vi) == k and vi in ends:
                live[ends[vi]] -= 1
    need = {e: peak[e] + 1 for e in peak}
    assert sum(need.values()) <= n_slots, f"need {need} > {n_slots} slots"
    ranges, lo = {}, 0
    for e in ("V", "G", "S"):
        ranges[e] = (lo, lo + need[e])
        lo += need[e]
    free = {e: list(range(r[1] - 1, r[0] - 1, -1)) for e, r in ranges.items()}
    slot_of = {}
    eng_of_slot = {}

    def ap_of(v):
        if isinstance(v, _VR):
            s = slot_of[int(v)]
            return ms_ap[:, s * C:(s + 1) * C]
        return v  # external AP

    for k, op_i in enumerate(order):
        kind, out, ins, extra = ir.ops[op_i]
        e = assign[op_i]
        if isinstance(out, _VR):
            assert free[e], f"scratch slots exhausted for engine {e}"
            slot = free[e].pop()
            slot_of[int(out)] = slot
            eng_of_slot[slot] = e
            out_ap = ms_ap[:, slot * C:(slot + 1) * C]
        else:
            out_ap = out
        in_aps = [ap_of(v) for v in ins]
        eng = {"V": nc.vector, "G": nc.gpsimd, "S": nc.scalar}[e]
        if kind == "tt":
            eng.tensor_tensor(out_ap, in_aps[0], in_aps[1], extra)
        elif kind == "ts":
            s1, op0, s2, op1 = extra
            if s2 is None:
                eng.tensor_scalar(out_ap, in_aps[0], s1, None, op0)
            else:
                eng.tensor_scalar(out_ap, in_aps[0], s1, s2, op0, op1)
        elif kind == "act":
            fn, scale, bias = extra
            if bias is None:
                nc.scalar.activation(out_ap, in_aps[0], fn, scale=scale)
            else:
                nc.scalar.activation(out_ap, in_aps[0], fn, scale=scale,
                                     bias=bias)
        else:
            raise ValueError(kind)
        for vi in {int(v) for v in ins if isinstance(v, _VR)}:
            if last_use.get(vi) == k:
                s = slot_of[vi]
                free[eng_of_slot[s]].append(s)
    return clock


def _record_math(ir, Ev, Rv, consts):
    """Record the whole rotation math on the IR. Ev/Rv are [128, 12, C] views
    (strided entry slices); consts maps name -> [128,1] const AP."""
    from concourse import mybir
    AF = mybir.ActivationFunctionType
    A_ = ir.A_

    Ae = [[Ev[:, 3 * a + b, :] for b in range(3)] for a in range(3)]
    me = [Ev[:, 9 + a, :] for a in range(3)]

    # S = A^T A (6 unique entries)
    Smat = {}
    for bi in range(3):
        for ci in range(bi, 3):
            Smat[(bi, ci)] = ir.dot3(Ae[0][bi], Ae[1][bi], Ae[2][bi],
                                     Ae[0][ci], Ae[1][ci], Ae[2][ci])

    def S(i, j):
        return Smat[(min(i, j), max(i, j))]

    q = ir.ts(ir.add(ir.add(S(0, 0), S(1, 1)), S(2, 2)), 1.0 / 3.0, A_.mult)
    P00 = ir.sub(S(0, 0), q)
    P11 = ir.sub(S(1, 1), q)
    P22 = ir.sub(S(2, 2), q)
    sq01 = ir.mul(S(0, 1), S(0, 1))
    sq02 = ir.mul(S(0, 2), S(0, 2))
    sq12 = ir.mul(S(1, 2), S(1, 2))
    diagsq = ir.add(ir.add(ir.mul(P00, P00), ir.mul(P11, P11)), ir.mul(P22, P22))
    offsq = ir.add(ir.add(sq01, sq02), sq12)
    p2v = ir.add(diagsq, ir.ts(offsq, 2.0, A_.mult))
    p2c = ir.ts(ir.ts(p2v, 1.0 / 6.0, A_.mult), 1e-30, A_.max)
    ln_p = ir.act(AF.Ln, p2c)
    pval = ir.act(AF.Exp, ln_p, scale=0.5)       # sqrt(p2c)
    pinv3 = ir.act(AF.Exp, ln_p, scale=-1.5)     # p2c^-1.5

    c0 = ir.sub(ir.mul(P11, P22), sq12)
    c1c = ir.sub(ir.mul(S(0, 1), P22), ir.mul(S(1, 2), S(0, 2)))
    c2c = ir.sub(ir.mul(S(0, 1), S(1, 2)), ir.mul(P11, S(0, 2)))
    detB = ir.add(ir.sub(ir.mul(P00, c0), ir.mul(S(0, 1), c1c)),
                  ir.mul(S(0, 2), c2c))
    rr = ir.ts(ir.mul(detB, pinv3), 0.5, A_.mult, 0.9999995, A_.min)
    rr = ir.ts(rr, -0.9999995, A_.max)

    omr = ir.ts(ir.mul(rr, rr), -1.0, A_.mult, 1.0, A_.add)
    rs = ir.rsqrt(omr)
    uu = ir.mul(rr, rs)
    # arctan(u) with range reduction — ACT Arctan domain is [-pi/2, pi/2]:
    # |u|<=1: a = arctan(|u|); |u|>1: pi/2 - arctan(1/|u|); then apply sign.
    au = ir.tt(A_.max, uu, ir.ts(uu, -1.0, A_.mult))      # |u|
    inv = ir.recip(ir.ts(au, 1e-30, A_.max))
    z = ir.tt(A_.min, au, inv)
    az = ir.act(AF.Arctan, z)
    dz = ir.ts(az, -1.0, A_.mult, float(np.pi / 2), A_.add)
    mge = ir.ts(au, 1.0, A_.is_ge)                        # |u| >= 1
    mle = ir.act(AF.Identity, mge, scale=-1.0, bias=consts["one"])  # 1 - that
    res_abs = ir.add(dz, ir.mul(mle, ir.sub(az, dz)))
    sgn_u = ir.ts(ir.ts(uu, 0.0, A_.is_ge), 2.0, A_.mult, -1.0, A_.add)
    at = ir.mul(res_abs, sgn_u)
    c1t = ir.act(AF.Sin, at, scale=1.0 / 3.0, bias=consts["pi3"])
    lam1 = ir.add(q, ir.ts(ir.mul(pval, c1t), 2.0, A_.mult))

    # v1 = best cross of rows of (S - lam1 I)
    D0 = ir.sub(S(0, 0), lam1)
    D1 = ir.sub(S(1, 1), lam1)
    D2 = ir.sub(S(2, 2), lam1)
    rows = [
        [D0, S(0, 1), S(0, 2)],
        [S(0, 1), D1, S(1, 2)],
        [S(0, 2), S(1, 2), D2],
    ]
    best, bn = None, None
    for (i, j) in [(0, 1), (0, 2)]:
        c = ir.cross3(rows[i], rows[j])
        n = ir.dot3(c[0], c[1], c[2], c[0], c[1], c[2])
        if best is None:
            best, bn = c, n
        else:
            m = ir.tt(A_.is_gt, n, bn)
            best = ir.blend3(m, c, best)
            bn = ir.add(bn, ir.mul(m, ir.sub(n, bn)))
    inv1 = ir.rsqrt(ir.ts(bn, 1e-37, A_.max))
    v1 = [ir.mul(best[0], inv1), ir.mul(best[1], inv1), ir.mul(best[2], inv1)]

    # (w2, w3): orthonormal complement of v1 via Householder columns.
    # H = I - h h^T/(1+a), h = v1 + s*e0, s = sign(v1x), a = s*v1x = |v1x|.
    sgn = ir.ts(ir.ts(v1[0], 0.0, A_.is_ge), 2.0, A_.mult, -1.0, A_.add)
    alpha = ir.mul(sgn, v1[0])
    denom = ir.ts(alpha, 1.0, A_.add)                     # 1 + |v1x| in [1,2]
    rden = ir.recip(denom)
    h0 = ir.add(v1[0], sgn)
    hyr = ir.mul(v1[1], rden)
    nhyr = ir.ts(hyr, -1.0, A_.mult)
    w2 = [ir.mul(h0, nhyr),
          ir.ts(ir.mul(v1[1], hyr), -1.0, A_.mult, 1.0, A_.add),
          ir.mul(v1[2], nhyr)]
    w3 = ir.cross3(v1, w2)

    # deflated 2x2 eigenproblem in span{w2, w3}; c2x via trace identity.
    Sw2 = [ir.dot3(S(bi, 0), S(bi, 1), S(bi, 2), w2[0], w2[1], w2[2])
           for bi in range(3)]
    a2x = ir.dot3(w2[0], w2[1], w2[2], Sw2[0], Sw2[1], Sw2[2])
    b2x = ir.dot3(Sw2[0], Sw2[1], Sw2[2], w3[0], w3[1], w3[2])
    trq = ir.act(AF.Identity, q, scale=3.0)
    c2x = ir.sub(trq, ir.add(lam1, a2x))

    half = ir.ts(ir.sub(a2x, c2x), 0.5, A_.mult)
    mpos = ir.ts(half, 0.0, A_.is_ge)
    sgn2 = ir.ts(mpos, 2.0, A_.mult, -1.0, A_.add)
    habs = ir.mul(sgn2, half)
    rad2 = ir.ts(ir.add(ir.mul(half, half), ir.mul(b2x, b2x)), 1e-37, A_.max)
    rad = ir.act(AF.Sqrt, rad2)
    pos = ir.ts(ir.add(habs, rad), 1e-37, A_.max)
    tq = ir.mul(ir.mul(b2x, ir.recip(pos)), sgn2)
    c2i = ir.rsqrt(ir.ts(ir.mul(tq, tq), 1.0, A_.add))
    s2i = ir.mul(tq, c2i)
    tb = ir.mul(tq, b2x)
    lamA = ir.add(a2x, tb)
    lamB = ir.sub(c2x, tb)
    mAB = ir.tt(A_.is_ge, lamA, lamB)
    vA = [ir.add(ir.mul(c2i, w2[i]), ir.mul(s2i, w3[i])) for i in range(3)]
    vB = [ir.sub(ir.mul(c2i, w3[i]), ir.mul(s2i, w2[i])) for i in range(3)]
    v2 = ir.blend3(mAB, vA, vB)
    v3 = ir.cross3(v1, v2)

    def Avec(v):
        return [ir.dot3(Ae[ai][0], Ae[ai][1], Ae[ai][2], v[0], v[1], v[2])
                for ai in range(3)]

    b1 = Avec(v1)
    n1 = ir.dot3(b1[0], b1[1], b1[2], b1[0], b1[1], b1[2])
    i1 = ir.rsqrt(ir.ts(n1, 1e-37, A_.max))
    u1 = [ir.mul(b1[i], i1) for i in range(3)]

    b2v = Avec(v2)
    dd = ir.dot3(u1[0], u1[1], u1[2], b2v[0], b2v[1], b2v[2])
    b2o = [ir.sub(b2v[i], ir.mul(dd, u1[i])) for i in range(3)]
    n2 = ir.dot3(b2o[0], b2o[1], b2o[2], b2o[0], b2o[1], b2o[2])
    i2 = ir.rsqrt(ir.ts(n2, 1e-37, A_.max))
    u2 = [ir.mul(b2o[i], i2) for i in range(3)]

    u3 = ir.cross3(u1, u2)

    us = [u1, u2, u3]
    vs = [v1, v2, v3]
    Re = [[None] * 3 for _ in range(3)]
    for ai in range(3):
        for bi in range(3):
            t1 = ir.mul(us[0][ai], vs[0][bi])
            t2 = ir.mul(us[1][ai], vs[1][bi])
            sgm = ir.add(t1, t2)
            t3 = ir.mul(us[2][ai], vs[2][bi])
            r = ir.add(sgm, t3)
            Re[ai][bi] = r
            ir.act(AF.Identity, r, out=Rv[:, 3 * ai + bi, :])

    mn = [ir.act(AF.Identity, me[i], scale=-1.0) for i in range(3)]
    for bi in range(3):
        t1 = ir.mul(mn[0], Re[0][bi])
        t2 = ir.mul(mn[1], Re[1][bi])
        sgm = ir.add(t1, t2)
        t3 = ir.mul(mn[2], Re[2][bi])
        ir.add(sgm, t3, out=Rv[:, 9 + bi, :])


# ----------------------------------------------------------------------------
# Bass program
# ----------------------------------------------------------------------------

def _split_multiwait(nc):
    """This walrus build encodes at most ONE semaphore wait per instruction,
    but Tile emits several. Split extras into standalone EventSemaphore
    (pure wait) instructions on the same engine, immediately before."""
    from concourse import mybir
    import bass_rust

    n_split = 0
    for fn in nc.m.functions:
        for blk in fn.blocks:
            new = []
            for ins in blk.instructions:
                si = ins.sync_info
                if si is not None and si.on_wait is not None and len(si.on_wait) > 1:
                    waits = list(si.on_wait)
                    for k, w in enumerate(waits[:-1]):
                        new.append(mybir.InstEventSemaphore(
                            name=f"{ins.name}-w{k}",
                            engine=ins.engine,
                            sync_info=bass_rust.SyncInfo(
                                on_wait=[w], on_update=[]),
                        ))
                        n_split += 1
                    ins.sync_info = bass_rust.SyncInfo(
                        on_wait=[waits[-1]],
                        on_update=list(si.on_update or []))
                new.append(ins)
            blk.instructions = new
    return n_split


def _build_program(ls=LS, n_slots=62, split_waits=True, prefetch=6):
    import concourse.bass as bass
    import concourse.tile as tile
    from concourse import mybir

    f32 = mybir.dt.float32
    bf16 = mybir.dt.bfloat16
    A_ = mybir.AluOpType
    AF = mybir.ActivationFunctionType

    nt = ls // 128
    C = nt

    nc = bass.Bass("TRN2", target_bir_lowering=False, debug=False)

    f32r = mybir.dt.float32r
    xgt_d = nc.dram_tensor("xgt", [192, ls], f32r, kind="ExternalInput").ap()
    xsep_d = nc.dram_tensor("xsep", [ls, 768], bf16, kind="ExternalInput").ap()
    w_d = nc.dram_tensor("wm", [192, 12], f32r, kind="ExternalInput").ap()
    id_d = nc.dram_tensor("ident", [12, 12], f32, kind="ExternalInput").ap()
    idb_d = nc.dram_tensor("identb", [128, 128], bf16, kind="ExternalInput").ap()
    out_d = nc.dram_tensor("out", [ls, 768], bf16, kind="ExternalOutput").ap()

    with tile.TileContext(nc) as tc:
        with (
            tc.tile_pool(name="wp", bufs=1) as wp,
            tc.tile_pool(name="gp_", bufs=1) as gpool,
            tc.tile_pool(name="ep", bufs=1) as ep,
            tc.tile_pool(name="xp", bufs=prefetch) as xp,
            tc.tile_pool(name="p2", bufs=2) as p2p,
            tc.tile_pool(name="op_", bufs=3) as opool,
        ):
            # ---------------- constants / weights ----------------
            identb = wp.tile([128, 128], bf16, tag="identb")
            nc.sync.dma_start(identb[:], idb_d)
            w0 = wp.tile([128, 12], f32r, tag="w0")
            w1 = wp.tile([64, 12], f32r, tag="w1")
            nc.sync.dma_start(w0[:], w_d[0:128, :])
            nc.sync.dma_start(w1[:], w_d[128:192, :])
            ident = wp.tile([12, 12], f32, tag="ident")
            nc.sync.dma_start(ident[:], id_d)

            ET = ep.tile([12, ls], f32, tag="ET")
            # E and R are ENTRY-MAJOR [128, e*nt + g]: math operands become
            # contiguous 64-column slices (DVE fast path, Pool software loop).
            E = ep.tile([128, nt * 12], f32, tag="E")
            R = ep.tile([128, nt * 12], f32, tag="R")
            MS = ep.tile([128, n_slots * C], f32, tag="MS")
            cst = ep.tile([128, 4], f32, tag="cst")
            nc.gpsimd.memset(cst[:, 0:1], float(np.pi / 3))
            nc.gpsimd.memset(cst[:, 1:2], float(np.pi / 2))
            nc.gpsimd.memset(cst[:, 2:3], 1.0)
            nc.gpsimd.memset(cst[:, 3:4], -1.0)
            consts = {"pi3": cst[:, 0:1], "pi2": cst[:, 1:2],
                      "one": cst[:, 2:3], "neg1": cst[:, 3:4]}
            Ev = E[:].rearrange("p (e g) -> p e g", e=12)
            Rv = R[:].rearrange("p (e g) -> p e g", e=12)
            Eg = E[:].rearrange("p (e g) -> p g e", e=12)

            # ---------------- phase 1: ET = W^T @ xgt (f32r), then PE ------
            # transposes back to E. Dummy PE matmuls absorb each DMA's
            # semaphore into the PE's observed clock (Matmult ISA slot holds
            # at most ONE wait). Phase-1 PSUM pools are scoped so their banks
            # free up for the apply-phase accumulator pool.
            psp = tc.alloc_tile_pool(name="ps", bufs=2, space="PSUM")
            pstp = tc.alloc_tile_pool(name="pst", bufs=2, space="PSUM")
            pss = tc.alloc_tile_pool(name="ps2", bufs=1, space="PSUM")
            ps_scr = pss.tile([128, 12], f32, tag="scr")
            nc.tensor.matmul(ps_scr[0:12, 0:12], w0[:, 0:12], w0[:],
                             start=True, stop=True)
            nc.tensor.matmul(ps_scr[0:12, 0:12], w1[:, 0:12], w1[:],
                             start=True, stop=True)
            n_strip = ls // 512
            slabs = []
            for s in range(n_strip):
                sl0 = gpool.tile([128, 512], f32r, tag=f"g0_{s}")
                sl1 = gpool.tile([64, 512], f32r, tag=f"g1_{s}")
                nc.sync.dma_start(sl0[:], xgt_d[0:128, s * 512:(s + 1) * 512])
                nc.sync.dma_start(sl1[:], xgt_d[128:192, s * 512:(s + 1) * 512])
                slabs.append((sl0, sl1))
            for s in range(n_strip):
                sl0, sl1 = slabs[s]
                nc.tensor.matmul(ps_scr[0:12, 0:12], sl0[:, 0:12], sl0[:, 0:12],
                                 start=True, stop=True)
                nc.tensor.matmul(ps_scr[0:12, 0:12], sl1[:, 0:12], sl1[:, 0:12],
                                 start=True, stop=True)
                psET = psp.tile([12, 512], f32, tag="psET")
                nc.tensor.matmul(psET[:], w0[:], sl0[:], start=True, stop=False)
                nc.tensor.matmul(psET[:], w1[:], sl1[:], start=False, stop=True)
                nc.scalar.copy(ET[:, s * 512:(s + 1) * 512], psET[:])
                if s % 2 == 1:
                    gb = s // 2      # transpose the 8 groups of strips s-1, s
                    psT = pstp.tile([128, 96], f32, tag="psT")
                    for k in range(8):
                        g = gb * 8 + k
                        nc.tensor.transpose(psT[:, k * 12:(k + 1) * 12],
                                            ET[:, g * 128:(g + 1) * 128],
                                            ident[:])
                    nc.scalar.copy(
                        Eg[:, gb * 8:(gb + 1) * 8, :],
                        psT[:].rearrange("p (g e) -> p g e", e=12))

            pss.release()
            pstp.release()
            psp.release()
            psop = tc.alloc_tile_pool(name="pso", bufs=2, space="PSUM")
            psc2 = tc.alloc_tile_pool(name="psc2", bufs=1, space="PSUM")
            scr2 = psc2.tile([16, 12], mybir.dt.float32, tag="scr2")

            # ---------------- phase 2: rotation math ----------------------
            ir = _MathIR(A_)
            _record_math(ir, Ev, Rv, consts)
            _emit_math(nc, ir, MS[:], C, n_slots)

            # ---------------- phase 3: apply (bf16) -----------------------
            n_grp = nt // 4
            xqs = []
            for grp in range(n_grp):
                xq = xp.tile([128, 4 * 768], bf16, tag="xq")
                src = xsep_d[grp * 512:(grp + 1) * 512, :].rearrange(
                    "(g p) c -> p g c", p=128)
                nc.sync.dma_start(xq[:].rearrange("p (g c) -> p g c", c=768), src)
                xqs.append(xq)
            # apply: DVE/ACT compute the 9 per-component products into
            # pair-sized bf16 tiles; the (otherwise idle) TensorEngine sums
            # P0+P1+P2 into PSUM via identity matmuls; Pool casts PSUM->bf16
            # SBUF; SP DMAs out. DVE never runs a wide add.
            for pr in range(nt // 2):
                P0 = p2p.tile([128, 1536], bf16, tag="P0")
                P1 = p2p.tile([128, 1536], bf16, tag="P1")
                P2 = p2p.tile([128, 1536], bf16, tag="P2")
                for half in range(2):
                    gg = pr * 2 + half
                    grp, t = gg // 4, gg % 4
                    xq = xqs[grp]
                    base = t * 768
                    hb = half * 768
                    for bi in range(3):
                        rcol0 = R[:, bi * nt + gg: bi * nt + gg + 1]
                        rcol1 = R[:, (3 + bi) * nt + gg: (3 + bi) * nt + gg + 1]
                        rcol2 = R[:, (6 + bi) * nt + gg: (6 + bi) * nt + gg + 1]
                        tncol = R[:, (9 + bi) * nt + gg: (9 + bi) * nt + gg + 1]
                        x0 = xq[:, base:base + 256]
                        x1 = xq[:, base + 256:base + 512]
                        x2 = xq[:, base + 512:base + 768]
                        p0s = P0[:, hb + bi * 256:hb + (bi + 1) * 256]
                        nc.scalar.activation(p0s, x0, AF.Identity,
                                             bias=tncol, scale=rcol0)
                        nc.vector.tensor_scalar(
                            P1[:, hb + bi * 256:hb + (bi + 1) * 256],
                            x1, rcol1, None, A_.mult)
                        nc.vector.tensor_scalar(
                            P2[:, hb + bi * 256:hb + (bi + 1) * 256],
                            x2, rcol2, None, A_.mult)
                # PE sums P0+P1+P2 into PSUM. The Matmult ISA slot holds at
                # most ONE semaphore wait, so three dummy matmuls absorb the
                # ACT-sem, DVE-sem and psO-WAR-sem into the PE's observed
                # clock first; the real matmuls then need no fresh waits.
                psO = psop.tile([128, 1536], f32, tag="psO")
                nc.tensor.matmul(scr2[:], P0[:, 1520:1536], P0[:, 1520:1532],
                                 start=True, stop=True, skip_group_check=True)
                nc.tensor.matmul(scr2[:], P2[:, 1520:1536], P2[:, 1520:1532],
                                 start=True, stop=True, skip_group_check=True)
                nc.tensor.matmul(psO[0:16, 0:16], identb[:, 0:16],
                                 identb[:, 0:16], start=True, stop=True,
                                 skip_group_check=True)
                for c0 in (0, 512, 1024):
                    nc.tensor.matmul(psO[:, c0:c0 + 512], identb[:],
                                     P0[:, c0:c0 + 512], start=True, stop=False)
                    nc.tensor.matmul(psO[:, c0:c0 + 512], identb[:],
                                     P1[:, c0:c0 + 512], start=False, stop=False)
                    nc.tensor.matmul(psO[:, c0:c0 + 512], identb[:],
                                     P2[:, c0:c0 + 512], start=False, stop=True)
                ot = opool.tile([128, 1536], bf16, tag="ot")
                for ci, c0 in enumerate((0, 512, 1024)):
                    on_v = ci == 0 or (ci == 1 and pr % 5 != 4)
                    if on_v:
                        nc.vector.tensor_scalar(ot[:, c0:c0 + 512],
                                                psO[:, c0:c0 + 512],
                                                1.0, None, A_.mult)
                    else:
                        nc.scalar.activation(ot[:, c0:c0 + 512],
                                             psO[:, c0:c0 + 512], AF.Identity)
                gg0 = pr * 2
                dst = out_d[gg0 * 128:(gg0 + 2) * 128, :].rearrange(
                    "(g p) c -> p g c", p=128)
                nc.sync.dma_start(dst, ot[:].rearrange(
                    "p (g c) -> p g c", c=768))
            psc2.release()
            psop.release()

    if split_waits:
        _split_multiwait(nc)
    return nc


# ----------------------------------------------------------------------------
# Host-side preparation
# ----------------------------------------------------------------------------

def _prep_inputs(x, ref_x, align_idx):
    import ml_dtypes
    BF16 = ml_dtypes.bfloat16
    x = np.asarray(x, dtype=F32)
    ref_x = np.asarray(ref_x)
    idx = np.asarray(align_idx).astype(np.int64)
    L = x.shape[0]

    ref64 = ref_x.astype(np.float64)
    ref_c = (ref64 - ref64.mean(0)).astype(F32)        # [64, 3]

    xg = x[:, idx, :]                                   # [L, 64, 3]
    xgt = np.ascontiguousarray(xg.reshape(L, 192).T)    # f32 [192, L]

    xsep = np.ascontiguousarray(
        x.transpose(0, 2, 1)).reshape(L, 768).astype(BF16)

    W = np.zeros((192, 12), dtype=F32)
    for a in range(3):
        rows = 3 * np.arange(N_ALIGN) + a
        for b in range(3):
            W[rows, 3 * a + b] = ref_c[:, b]
        W[rows, 9 + a] = F32(1.0 / N_ALIGN)
    return xgt, xsep, W


# ----------------------------------------------------------------------------
# Runner: jit once, reuse
# ----------------------------------------------------------------------------

class _Runner:
    def __init__(self):
        import jax

        self.jax = jax
        self.nc = _build_program(LS)
        self._build_exec()

    def _build_exec(self):
        import jax
        from jax.sharding import Mesh, PartitionSpec
        from jax.experimental.shard_map import shard_map
        from concourse import mybir
        from concourse.bass2jax import (_bass_exec_p, install_neuronx_cc_hook,
                                        partition_id_tensor)

        install_neuronx_cc_hook()
        # surface compile-hook exceptions (PJRT swallows them)
        try:
            import libneuronxla
            import traceback
            if not getattr(libneuronxla, "_ant_logged_cc", False):
                _orig_cc = libneuronxla.neuronx_cc

                def _logged_cc(*a, **k):
                    try:
                        return _orig_cc(*a, **k)
                    except BaseException:
                        traceback.print_exc()
                        raise

                libneuronxla.neuronx_cc = _logged_cc
                libneuronxla._ant_logged_cc = True
        except ImportError:
            pass
        nc = self.nc

        part_name = (nc.partition_id_tensor.name
                     if nc.partition_id_tensor else None)
        in_names, out_names, out_avals = [], [], []
        for alloc in nc.m.functions[0].allocations:
            if not isinstance(alloc, mybir.MemoryLocationSet):
                continue
            name = alloc.memorylocations[0].name
            if alloc.kind == "ExternalInput":
                if name != part_name:
                    in_names.append(name)
            elif alloc.kind == "ExternalOutput":
                shape = tuple(alloc.tensor_shape)
                dtype = mybir.dt.np(alloc.dtype)
                out_names.append(name)
                out_avals.append(jax.core.ShapedArray(shape, dtype))
        self.in_names = list(in_names)
        self.out_names = list(out_names)
        n_params = len(in_names)
        all_names = in_names + out_names
        if part_name is not None:
            all_names = all_names + [part_name]

        def _body(*args):
            operands = list(args)
            if part_name is not None:
                operands.append(partition_id_tensor())
            outs = _bass_exec_p.bind(
                *operands,
                out_avals=tuple(out_avals),
                in_names=tuple(all_names),
                out_names=tuple(out_names),
                lowering_input_output_aliases=(),
                sim_require_finite=True,
                sim_require_nnan=True,
                nc=nc,
            )
            return tuple(outs)

        devices = jax.devices()[:N_CORES]
        mesh = Mesh(np.asarray(devices), ("core",))
        n_outs = len(out_names)
        in_specs = (PartitionSpec("core"),) * (n_params + n_outs)
        out_specs = (PartitionSpec("core"),) * n_outs
        self._fn = jax.jit(
            shard_map(_body, mesh=mesh, in_specs=in_specs,
                      out_specs=out_specs, check_rep=False),
            keep_unused=True,
        )
        self._zeros = [
            np.zeros((N_CORES * av.shape[0], *av.shape[1:]), av.dtype)
            for av in out_avals
        ]

    def stage(self, x, ref_x, align_idx):
        import ml_dtypes
        xgt, xsep, W = _prep_inputs(x, ref_x, align_idx)
        per_name = {
            "xgt": np.concatenate(
                [xgt[:, c * LS:(c + 1) * LS] for c in range(N_CORES)], axis=0),
            "xsep": xsep,
            "wm": np.concatenate([W] * N_CORES, axis=0),
            "ident": np.concatenate(
                [np.eye(12, dtype=F32)] * N_CORES, axis=0),
            "identb": np.concatenate(
                [np.eye(128).astype(ml_dtypes.bfloat16)] * N_CORES, axis=0),
        }
        args = [per_name[n] for n in self.in_names] + list(self._zeros)
        return [self.jax.device_put(a) for a in args]

    def run_staged(self, staged):
        return self._fn(*staged)

    def run(self, x, ref_x, align_idx):
        staged = self.stage(x, ref_x, align_idx)
        outs = self.run_staged(staged)
        out = np.asarray(outs[self.out_names.index("out")]).astype(np.float32)
        L = out.shape[0]
        return np.ascontiguousarray(
            out.reshape(L, 3, N_INP).transpose(0, 2, 1))


def _get_runner():
    global _RUNNER
    if _RUNNER is None:
        _RUNNER = _Runner()
    return _RUNNER


def kernel(x, ref_x, align_idx):
    runner = _get_runner()
    return runner.run(x, ref_x, align_idx).astype(np.float32)


if __name__ == "__main__":
    nc = _build_program(LS)
    print("built ok")


# revision 46
# speedup vs baseline: 1.2714x; 1.1684x over previous
"""Trainium2 Bass kernel for nn_AlignmentLayer (Kabsch alignment of L frames).

Strategy (pure data parallel over 8 NeuronCores, L/8 = 8192 frames per core):

Host-side (numpy, cheap layout work only):
  - ref_c = ref_x - mean(ref_x); gather xg = x[:, align_idx, :]  (align_idx is
    a host-known constant input, so the gather folds into data layout).
  - xgt: gathered atoms pre-transposed to [192, L] f32 so phase 1 needs
    zero on-chip transposes.
  - xsep: x in component-major layout [L, 3, 256] bf16 so phase-3 tensor ops
    are contiguous; output produced component-major bf16 and unpacked on host.
  - W: [192, 12] f32 weights mapping gathered rows to the 9 entries of
    A = xg^T @ ref_c and the 3 entries of the centroid x_c.

Device (per core), three phases:
  1. PE matmuls, weight-stationary: ET[12, ls] = W^T @ xgt in 512-frame
     strips (f32 for exact E — bf16 E perturbs near-singular frames), then
     PE-transposed back to E[128, nt*12] via identity matmuls.
  2. Math (DVE + Pool + ACT, batched [128, 64] ops): SVD-free Kabsch
     rotation. S = A^T A; lambda1 via trigonometric cubic (arctan+sin);
     v1 = best cross product of rows of (S - lambda1 I); (v2, v3) from a
     deflated 2x2 eigenproblem in the Householder complement of v1;
     u_i = normalize(A v_i); u3 = u1 x u2; R = sum u_i v_i^T; tneg = -x_c R.
     rsqrt/recip computed as Exp(-c*Ln(x)) on ACT; a greedy list scheduler
     splits the op DAG across DVE and Pool.
  3. Apply (bf16): per 128-frame tile and component b, products
     P0 = x0*R0b + tneg_b (ACT), P1 = x1*R1b, P2 = x2*R2b (DVE ts), then
     two 768-wide DVE adds produce the output tile.
"""

import numpy as np

L_FULL = 65536
N_INP = 256
N_ALIGN = 64
N_CORES = 8
LS = L_FULL // N_CORES          # frames per core
NT = LS // 128                  # 128-frame tiles per core (64)
F32 = np.float32

_RUNNER = None


# ----------------------------------------------------------------------------
# Math IR: record ops on virtual registers; a greedy list scheduler assigns
# each op to DVE ("V") or Pool ("G") (ACT ops pinned to "S"), then emission
# uses per-engine linear-scan slot allocation into one scratch tensor.
# ----------------------------------------------------------------------------

class _VR(int):
    """Virtual register id."""


# measured per-op engine costs at [128, 64] f32 (ns)
_COST = {
    ("tt", "V"): 150, ("tt", "G"): 300,
    ("ts", "V"): 115, ("ts", "G"): 260,
}
_ACT_COST = {"Ln": 240, "Exp": 350}
_XENG_NS = 250        # cross-engine result handoff penalty
_V_BIAS = 1.0         # apply follows math serially, so just balance math wall
_REBAL_NS = 800       # affinity hysteresis: rebalance only past this drift


class _MathIR:
    def __init__(self, alu):
        self.A_ = alu
        self.ops = []           # (kind, out, ins, extra)
        self.n = 0

    def _rec(self, kind, ins, extra=None, out=None):
        if out is None:
            out = _VR(self.n)
            self.n += 1
        self.ops.append((kind, out, list(ins), extra))
        return out

    def tt(self, op, a, b, out=None):
        return self._rec("tt", [a, b], op, out)

    def mul(self, a, b, out=None):
        return self.tt(self.A_.mult, a, b, out)

    def add(self, a, b, out=None):
        return self.tt(self.A_.add, a, b, out)

    def sub(self, a, b, out=None):
        return self.tt(self.A_.subtract, a, b, out)

    def ts(self, a, s1, op0, s2=None, op1=None, out=None):
        return self._rec("ts", [a], (float(s1), op0,
                                     None if s2 is None else float(s2), op1), out)

    def act(self, fn, a, scale=1.0, bias=None, out=None):
        return self._rec("act", [a], (fn, scale, bias), out)

    def rsqrt(self, nval):
        """1/sqrt(n) = Exp(-0.5*Ln(n)) on ACT (n must be > 0)."""
        from concourse import mybir
        AF = mybir.ActivationFunctionType
        ln = self.act(AF.Ln, nval)
        return self.act(AF.Exp, ln, scale=-0.5)

    def recip(self, nval):
        """1/n = Exp(-Ln(n)) on ACT (n must be > 0)."""
        from concourse import mybir
        AF = mybir.ActivationFunctionType
        ln = self.act(AF.Ln, nval)
        return self.act(AF.Exp, ln, scale=-1.0)

    def dot3(self, ax, ay, az, bx, by, bz):
        t1 = self.mul(ax, bx)
        t2 = self.mul(ay, by)
        s = self.add(t1, t2)
        t3 = self.mul(az, bz)
        return self.add(s, t3)

    def cross3(self, a, b):
        cx = self.sub(self.mul(a[1], b[2]), self.mul(a[2], b[1]))
        cy = self.sub(self.mul(a[2], b[0]), self.mul(a[0], b[2]))
        cz = self.sub(self.mul(a[0], b[1]), self.mul(a[1], b[0]))
        return [cx, cy, cz]

    def blend3(self, m, a, b):
        out = []
        for i in range(3):
            d = self.sub(a[i], b[i])
            out.append(self.add(b[i], self.mul(m, d)))
        return out


_RAW_LAT = 100        # same-engine RAW result latency (SBUF write ack)


def _schedule_math(ir):
    """Latency-aware list scheduling: all tensor ops on V, acts on S, and the
    EMISSION ORDER is chosen so dependent ops are spaced apart (back-to-back
    RAW chains pay the DVE write-ack latency). Returns (order, assign, clock).
    """
    n = len(ir.ops)
    eng, cost = [], []
    for kind, out, ins, extra in ir.ops:
        if kind == "act":
            fname = getattr(extra[0], "name", str(extra[0]))
            eng.append("S")
            cost.append(_ACT_COST.get(fname, 440))
        else:
            eng.append("V")
            cost.append(_COST[(kind, "V")])

    # dependency edges via vregs
    producer = {}
    deps = [[] for _ in range(n)]
    users = [[] for _ in range(n)]
    for i, (kind, out, ins, extra) in enumerate(ir.ops):
        for v in ins:
            if isinstance(v, _VR) and int(v) in producer:
                p = producer[int(v)]
                deps[i].append(p)
                users[p].append(i)
        if isinstance(out, _VR):
            producer[int(out)] = i

    # height = critical-path length to any sink
    height = [0] * n
    for i in range(n - 1, -1, -1):
        h = cost[i]
        for u in users[i]:
            h = max(h, cost[i] + height[u])
        height[i] = h

    indeg = [len(set(deps[i])) for i in range(n)]
    ready = [i for i in range(n) if indeg[i] == 0]
    clock = {"V": 0.0, "S": 0.0, "G": 0.0}
    fin = [0.0] * n
    done_deps = [set() for _ in range(n)]
    order = []
    import heapq
    while ready:
        # earliest feasible start per candidate
        best, best_key = None, None
        for i in ready:
            e = eng[i]
            est = clock[e]
            for p in set(deps[i]):
                lat = _RAW_LAT if eng[p] == e else _XENG_NS
                est = max(est, fin[p] + lat)
            stall = est - clock[e]
            key = (stall, -height[i])
            if best_key is None or key < best_key:
                best, best_key, best_est = i, key, est
        i = best
        ready.remove(i)
        e = eng[i]
        fin[i] = best_est + cost[i]
        clock[e] = fin[i]
        order.append(i)
        for u in users[i]:
            done_deps[u].add(i)
            if len(done_deps[u]) == len(set(deps[u])) and u not in ready \
                    and u not in order:
                ready.append(u)
    assert len(order) == n
    return order, eng, clock


def _emit_math(nc, ir, ms_ap, C, n_slots):
    """Emit recorded IR in the latency-aware schedule order. Vreg v lives in
    ms_ap[:, slot*C:(slot+1)*C]; slots partitioned per engine so WAR reuse
    stays engine-local."""
    order, assign, clock = _schedule_math(ir)

    # last use position in the EMISSION order
    pos = {op_i: k for k, op_i in enumerate(order)}
    last_use = {}
    for i, (kind, out, ins, extra) in enumerate(ir.ops):
        for v in ins:
            if isinstance(v, _VR):
                last_use[int(v)] = max(last_use.get(int(v), -1), pos[i])

    # per-engine slot ranges sized from peak live-value demand (emission order)
    peak = {"V": 0, "G": 0, "S": 0}
    live = {"V": 0, "G": 0, "S": 0}
    ends = {}
    for k, op_i in enumerate(order):
        kind, out, ins, extra = ir.ops[op_i]
        e = assign[op_i]
        if isinstance(out, _VR):
            live[e] += 1
            peak[e] = max(peak[e], live[e])
            ends[int(out)] = e
        for vi in {int(v) for v in ins if isinstance(v, _VR)}:
            if last_use.get(vi) == k and vi in ends:
                live[ends[vi]] -= 1
    need = {e: peak[e] + 1 for e in peak}
    assert sum(need.values()) <= n_slots, f"need {need} > {n_slots} slots"
    ranges, lo = {}, 0
    for e in ("V", "G", "S"):
        ranges[e] = (lo, lo + need[e])
        lo += need[e]
    free = {e: list(range(r[1] - 1, r[0] - 1, -1)) for e, r in ranges.items()}
    slot_of = {}
    eng_of_slot = {}

    def ap_of(v):
        if isinstance(v, _VR):
            s = slot_of[int(v)]
            return ms_ap[:, s * C:(s + 1) * C]
        return v  # external AP

    for k, op_i in enumerate(order):
        kind, out, ins, extra = ir.ops[op_i]
        e = assign[op_i]
        if isinstance(out, _VR):
            assert free[e], f"scratch slots exhausted for engine {e}"
            slot = free[e].pop()
            slot_of[int(out)] = slot
            eng_of_slot[slot] = e
            out_ap = ms_ap[:, slot * C:(slot + 1) * C]
        else:
            out_ap = out
        in_aps = [ap_of(v) for v in ins]
        eng = {"V": nc.vector, "G": nc.gpsimd, "S": nc.scalar}[e]
        if kind == "tt":
            eng.tensor_tensor(out_ap, in_aps[0], in_aps[1], extra)
        elif kind == "ts":
            s1, op0, s2, op1 = extra
            if s2 is None:
                eng.tensor_scalar(out_ap, in_aps[0], s1, None, op0)
            else:
                eng.tensor_scalar(out_ap, in_aps[0], s1, s2, op0, op1)
        elif kind == "act":
            fn, scale, bias = extra
            if bias is None:
                nc.scalar.activation(out_ap, in_aps[0], fn, scale=scale)
            else:
                nc.scalar.activation(out_ap, in_aps[0], fn, scale=scale,
                                     bias=bias)
        else:
            raise ValueError(kind)
        for vi in {int(v) for v in ins if isinstance(v, _VR)}:
            if last_use.get(vi) == k:
                s = slot_of[vi]
                free[eng_of_slot[s]].append(s)
    return clock


def _record_math(ir, Ev, Rv, consts):
    """Record the whole rotation math on the IR. Ev/Rv are [128, 12, C] views
    (strided entry slices); consts maps name -> [128,1] const AP."""
    from concourse import mybir
    AF = mybir.ActivationFunctionType
    A_ = ir.A_

    Ae = [[Ev[:, 3 * a + b, :] for b in range(3)] for a in range(3)]
    me = [Ev[:, 9 + a, :] for a in range(3)]

    # S = A^T A (6 unique entries)
    Smat = {}
    for bi in range(3):
        for ci in range(bi, 3):
            Smat[(bi, ci)] = ir.dot3(Ae[0][bi], Ae[1][bi], Ae[2][bi],
                                     Ae[0][ci], Ae[1][ci], Ae[2][ci])

    def S(i, j):
        return Smat[(min(i, j), max(i, j))]

    q = ir.ts(ir.add(ir.add(S(0, 0), S(1, 1)), S(2, 2)), 1.0 / 3.0, A_.mult)
    P00 = ir.sub(S(0, 0), q)
    P11 = ir.sub(S(1, 1), q)
    P22 = ir.sub(S(2, 2), q)
    sq01 = ir.mul(S(0, 1), S(0, 1))
    sq02 = ir.mul(S(0, 2), S(0, 2))
    sq12 = ir.mul(S(1, 2), S(1, 2))
    diagsq = ir.add(ir.add(ir.mul(P00, P00), ir.mul(P11, P11)), ir.mul(P22, P22))
    offsq = ir.add(ir.add(sq01, sq02), sq12)
    p2v = ir.add(diagsq, ir.ts(offsq, 2.0, A_.mult))
    p2c = ir.ts(ir.ts(p2v, 1.0 / 6.0, A_.mult), 1e-30, A_.max)
    ln_p = ir.act(AF.Ln, p2c)
    pval = ir.act(AF.Exp, ln_p, scale=0.5)       # sqrt(p2c)
    pinv3 = ir.act(AF.Exp, ln_p, scale=-1.5)     # p2c^-1.5

    c0 = ir.sub(ir.mul(P11, P22), sq12)
    c1c = ir.sub(ir.mul(S(0, 1), P22), ir.mul(S(1, 2), S(0, 2)))
    c2c = ir.sub(ir.mul(S(0, 1), S(1, 2)), ir.mul(P11, S(0, 2)))
    detB = ir.add(ir.sub(ir.mul(P00, c0), ir.mul(S(0, 1), c1c)),
                  ir.mul(S(0, 2), c2c))
    rr = ir.ts(ir.mul(detB, pinv3), 0.5, A_.mult, 0.9999995, A_.min)
    rr = ir.ts(rr, -0.9999995, A_.max)

    omr = ir.ts(ir.mul(rr, rr), -1.0, A_.mult, 1.0, A_.add)
    rs = ir.rsqrt(omr)
    uu = ir.mul(rr, rs)
    # arctan(u) with range reduction — ACT Arctan domain is [-pi/2, pi/2]:
    # |u|<=1: a = arctan(|u|); |u|>1: pi/2 - arctan(1/|u|); then apply sign.
    au = ir.tt(A_.max, uu, ir.ts(uu, -1.0, A_.mult))      # |u|
    inv = ir.recip(ir.ts(au, 1e-30, A_.max))
    z = ir.tt(A_.min, au, inv)
    az = ir.act(AF.Arctan, z)
    dz = ir.ts(az, -1.0, A_.mult, float(np.pi / 2), A_.add)
    mge = ir.ts(au, 1.0, A_.is_ge)                        # |u| >= 1
    mle = ir.act(AF.Identity, mge, scale=-1.0, bias=consts["one"])  # 1 - that
    res_abs = ir.add(dz, ir.mul(mle, ir.sub(az, dz)))
    sgn_u = ir.ts(ir.ts(uu, 0.0, A_.is_ge), 2.0, A_.mult, -1.0, A_.add)
    at = ir.mul(res_abs, sgn_u)
    c1t = ir.act(AF.Sin, at, scale=1.0 / 3.0, bias=consts["pi3"])
    lam1 = ir.add(q, ir.ts(ir.mul(pval, c1t), 2.0, A_.mult))

    # v1 = best cross of rows of (S - lam1 I)
    D0 = ir.sub(S(0, 0), lam1)
    D1 = ir.sub(S(1, 1), lam1)
    D2 = ir.sub(S(2, 2), lam1)
    rows = [
        [D0, S(0, 1), S(0, 2)],
        [S(0, 1), D1, S(1, 2)],
        [S(0, 2), S(1, 2), D2],
    ]
    best, bn = None, None
    for (i, j) in [(0, 1), (0, 2)]:
        c = ir.cross3(rows[i], rows[j])
        n = ir.dot3(c[0], c[1], c[2], c[0], c[1], c[2])
        if best is None:
            best, bn = c, n
        else:
            m = ir.tt(A_.is_gt, n, bn)
            best = ir.blend3(m, c, best)
            bn = ir.add(bn, ir.mul(m, ir.sub(n, bn)))
    inv1 = ir.rsqrt(ir.ts(bn, 1e-37, A_.max))
    v1 = [ir.mul(best[0], inv1), ir.mul(best[1], inv1), ir.mul(best[2], inv1)]

    # (w2, w3): orthonormal complement of v1 via Householder columns.
    # H = I - h h^T/(1+a), h = v1 + s*e0, s = sign(v1x), a = s*v1x = |v1x|.
    sgn = ir.ts(ir.ts(v1[0], 0.0, A_.is_ge), 2.0, A_.mult, -1.0, A_.add)
    alpha = ir.mul(sgn, v1[0])
    denom = ir.ts(alpha, 1.0, A_.add)                     # 1 + |v1x| in [1,2]
    rden = ir.recip(denom)
    h0 = ir.add(v1[0], sgn)
    hyr = ir.mul(v1[1], rden)
    nhyr = ir.ts(hyr, -1.0, A_.mult)
    w2 = [ir.mul(h0, nhyr),
          ir.ts(ir.mul(v1[1], hyr), -1.0, A_.mult, 1.0, A_.add),
          ir.mul(v1[2], nhyr)]
    w3 = ir.cross3(v1, w2)

    # deflated 2x2 eigenproblem in span{w2, w3}; c2x via trace identity.
    Sw2 = [ir.dot3(S(bi, 0), S(bi, 1), S(bi, 2), w2[0], w2[1], w2[2])
           for bi in range(3)]
    a2x = ir.dot3(w2[0], w2[1], w2[2], Sw2[0], Sw2[1], Sw2[2])
    b2x = ir.dot3(Sw2[0], Sw2[1], Sw2[2], w3[0], w3[1], w3[2])
    trq = ir.act(AF.Identity, q, scale=3.0)
    c2x = ir.sub(trq, ir.add(lam1, a2x))

    half = ir.ts(ir.sub(a2x, c2x), 0.5, A_.mult)
    mpos = ir.ts(half, 0.0, A_.is_ge)
    sgn2 = ir.ts(mpos, 2.0, A_.mult, -1.0, A_.add)
    habs = ir.mul(sgn2, half)
    rad2 = ir.ts(ir.add(ir.mul(half, half), ir.mul(b2x, b2x)), 1e-37, A_.max)
    rad = ir.act(AF.Sqrt, rad2)
    pos = ir.ts(ir.add(habs, rad), 1e-37, A_.max)
    tq = ir.mul(ir.mul(b2x, ir.recip(pos)), sgn2)
    c2i = ir.rsqrt(ir.ts(ir.mul(tq, tq), 1.0, A_.add))
    s2i = ir.mul(tq, c2i)
    tb = ir.mul(tq, b2x)
    lamA = ir.add(a2x, tb)
    lamB = ir.sub(c2x, tb)
    mAB = ir.tt(A_.is_ge, lamA, lamB)
    vA = [ir.add(ir.mul(c2i, w2[i]), ir.mul(s2i, w3[i])) for i in range(3)]
    vB = [ir.sub(ir.mul(c2i, w3[i]), ir.mul(s2i, w2[i])) for i in range(3)]
    v2 = ir.blend3(mAB, vA, vB)
    v3 = ir.cross3(v1, v2)

    def Avec(v):
        return [ir.dot3(Ae[ai][0], Ae[ai][1], Ae[ai][2], v[0], v[1], v[2])
                for ai in range(3)]

    b1 = Avec(v1)
    n1 = ir.dot3(b1[0], b1[1], b1[2], b1[0], b1[1], b1[2])
    i1 = ir.rsqrt(ir.ts(n1, 1e-37, A_.max))
    u1 = [ir.mul(b1[i], i1) for i in range(3)]

    b2v = Avec(v2)
    dd = ir.dot3(u1[0], u1[1], u1[2], b2v[0], b2v[1], b2v[2])
    b2o = [ir.sub(b2v[i], ir.mul(dd, u1[i])) for i in range(3)]
    n2 = ir.dot3(b2o[0], b2o[1], b2o[2], b2o[0], b2o[1], b2o[2])
    i2 = ir.rsqrt(ir.ts(n2, 1e-37, A_.max))
    u2 = [ir.mul(b2o[i], i2) for i in range(3)]

    u3 = ir.cross3(u1, u2)

    us = [u1, u2, u3]
    vs = [v1, v2, v3]
    Re = [[None] * 3 for _ in range(3)]
    for ai in range(3):
        for bi in range(3):
            t1 = ir.mul(us[0][ai], vs[0][bi])
            t2 = ir.mul(us[1][ai], vs[1][bi])
            sgm = ir.add(t1, t2)
            t3 = ir.mul(us[2][ai], vs[2][bi])
            r = ir.add(sgm, t3)
            Re[ai][bi] = r
            ir.act(AF.Identity, r, out=Rv[:, 3 * ai + bi, :])

    mn = [ir.act(AF.Identity, me[i], scale=-1.0) for i in range(3)]
    for bi in range(3):
        t1 = ir.mul(mn[0], Re[0][bi])
        t2 = ir.mul(mn[1], Re[1][bi])
        sgm = ir.add(t1, t2)
        t3 = ir.mul(mn[2], Re[2][bi])
        ir.add(sgm, t3, out=Rv[:, 9 + bi, :])


# ----------------------------------------------------------------------------
# Bass program
# ----------------------------------------------------------------------------

def _split_multiwait(nc):
    """This walrus build encodes at most ONE semaphore wait per instruction,
    but Tile emits several. Split extras into standalone EventSemaphore
    (pure wait) instructions on the same engine, immediately before."""
    from concourse import mybir
    import bass_rust

    n_split = 0
    for fn in nc.m.functions:
        for blk in fn.blocks:
            new = []
            for ins in blk.instructions:
                si = ins.sync_info
                if si is not None and si.on_wait is not None and len(si.on_wait) > 1:
                    waits = list(si.on_wait)
                    for k, w in enumerate(waits[:-1]):
                        new.append(mybir.InstEventSemaphore(
                            name=f"{ins.name}-w{k}",
                            engine=ins.engine,
                            sync_info=bass_rust.SyncInfo(
                                on_wait=[w], on_update=[]),
                        ))
                        n_split += 1
                    ins.sync_info = bass_rust.SyncInfo(
                        on_wait=[waits[-1]],
                        on_update=list(si.on_update or []))
                new.append(ins)
            blk.instructions = new
    return n_split


def _build_program(ls=LS, n_slots=62, split_waits=True, prefetch=6):
    import concourse.bass as bass
    import concourse.tile as tile
    from concourse import mybir

    f32 = mybir.dt.float32
    bf16 = mybir.dt.bfloat16
    A_ = mybir.AluOpType
    AF = mybir.ActivationFunctionType

    nt = ls // 128
    C = nt

    nc = bass.Bass("TRN2", target_bir_lowering=False, debug=False)

    f32r = mybir.dt.float32r
    xgt_d = nc.dram_tensor("xgt", [192, ls], f32r, kind="ExternalInput").ap()
    xsep_d = nc.dram_tensor("xsep", [ls, 768], bf16, kind="ExternalInput").ap()
    w_d = nc.dram_tensor("wm", [192, 12], f32r, kind="ExternalInput").ap()
    id_d = nc.dram_tensor("ident", [12, 12], f32, kind="ExternalInput").ap()
    idb_d = nc.dram_tensor("identb", [128, 128], bf16, kind="ExternalInput").ap()
    out_d = nc.dram_tensor("out", [ls, 768], bf16, kind="ExternalOutput").ap()

    with tile.TileContext(nc) as tc:
        with (
            tc.tile_pool(name="wp", bufs=1) as wp,
            tc.tile_pool(name="gp_", bufs=1) as gpool,
            tc.tile_pool(name="ep", bufs=1) as ep,
            tc.tile_pool(name="xp", bufs=prefetch) as xp,
            tc.tile_pool(name="p2", bufs=6) as p2p,
            tc.tile_pool(name="op_", bufs=3) as opool,
        ):
            # ---------------- constants / weights ----------------
            identb = wp.tile([128, 128], bf16, tag="identb")
            nc.sync.dma_start(identb[:], idb_d)
            w0 = wp.tile([128, 12], f32r, tag="w0")
            w1 = wp.tile([64, 12], f32r, tag="w1")
            nc.sync.dma_start(w0[:], w_d[0:128, :])
            nc.sync.dma_start(w1[:], w_d[128:192, :])
            ident = wp.tile([12, 12], f32, tag="ident")
            nc.sync.dma_start(ident[:], id_d)

            ET = ep.tile([12, ls], f32, tag="ET")
            # E and R are ENTRY-MAJOR [128, e*nt + g]: math operands become
            # contiguous 64-column slices (DVE fast path, Pool software loop).
            E = ep.tile([128, nt * 12], f32, tag="E")
            R = ep.tile([128, nt * 12], f32, tag="R")
            MS = ep.tile([128, n_slots * C], f32, tag="MS")
            cst = ep.tile([128, 4], f32, tag="cst")
            nc.gpsimd.memset(cst[:, 0:1], float(np.pi / 3))
            nc.gpsimd.memset(cst[:, 1:2], float(np.pi / 2))
            nc.gpsimd.memset(cst[:, 2:3], 1.0)
            nc.gpsimd.memset(cst[:, 3:4], -1.0)
            consts = {"pi3": cst[:, 0:1], "pi2": cst[:, 1:2],
                      "one": cst[:, 2:3], "neg1": cst[:, 3:4]}
            Ev = E[:].rearrange("p (e g) -> p e g", e=12)
            Rv = R[:].rearrange("p (e g) -> p e g", e=12)
            Eg = E[:].rearrange("p (e g) -> p g e", e=12)

            # ---------------- phase 1: ET = W^T @ xgt (f32r), then PE ------
            # transposes back to E. Dummy PE matmuls absorb each DMA's
            # semaphore into the PE's observed clock (Matmult ISA slot holds
            # at most ONE wait). Phase-1 PSUM pools are scoped so their banks
            # free up for the apply-phase accumulator pool.
            psp = tc.alloc_tile_pool(name="ps", bufs=2, space="PSUM")
            pstp = tc.alloc_tile_pool(name="pst", bufs=2, space="PSUM")
            pss = tc.alloc_tile_pool(name="ps2", bufs=1, space="PSUM")
            ps_scr = pss.tile([128, 12], f32, tag="scr")
            nc.tensor.matmul(ps_scr[0:12, 0:12], w0[:, 0:12], w0[:],
                             start=True, stop=True)
            nc.tensor.matmul(ps_scr[0:12, 0:12], w1[:, 0:12], w1[:],
                             start=True, stop=True)
            n_strip = ls // 512
            slabs = []
            for s in range(n_strip):
                sl0 = gpool.tile([128, 512], f32r, tag=f"g0_{s}")
                sl1 = gpool.tile([64, 512], f32r, tag=f"g1_{s}")
                nc.sync.dma_start(sl0[:], xgt_d[0:128, s * 512:(s + 1) * 512])
                nc.sync.dma_start(sl1[:], xgt_d[128:192, s * 512:(s + 1) * 512])
                slabs.append((sl0, sl1))
            for s in range(n_strip):
                sl0, sl1 = slabs[s]
                nc.tensor.matmul(ps_scr[0:12, 0:12], sl0[:, 0:12], sl0[:, 0:12],
                                 start=True, stop=True)
                nc.tensor.matmul(ps_scr[0:12, 0:12], sl1[:, 0:12], sl1[:, 0:12],
                                 start=True, stop=True)
                psET = psp.tile([12, 512], f32, tag="psET")
                nc.tensor.matmul(psET[:], w0[:], sl0[:], start=True, stop=False)
                nc.tensor.matmul(psET[:], w1[:], sl1[:], start=False, stop=True)
                nc.scalar.copy(ET[:, s * 512:(s + 1) * 512], psET[:])
                if s % 2 == 1:
                    gb = s // 2      # transpose the 8 groups of strips s-1, s
                    psT = pstp.tile([128, 96], f32, tag="psT")
                    for k in range(8):
                        g = gb * 8 + k
                        nc.tensor.transpose(psT[:, k * 12:(k + 1) * 12],
                                            ET[:, g * 128:(g + 1) * 128],
                                            ident[:])
                    nc.scalar.copy(
                        Eg[:, gb * 8:(gb + 1) * 8, :],
                        psT[:].rearrange("p (g e) -> p g e", e=12))

            pss.release()
            pstp.release()
            psp.release()
            psop = tc.alloc_tile_pool(name="pso", bufs=2, space="PSUM")
            psc2 = tc.alloc_tile_pool(name="psc2", bufs=1, space="PSUM")
            scr2 = psc2.tile([16, 12], mybir.dt.float32, tag="scr2")

            # ---------------- phase 2: rotation math ----------------------
            ir = _MathIR(A_)
            _record_math(ir, Ev, Rv, consts)
            _emit_math(nc, ir, MS[:], C, n_slots)

            # ---------------- phase 3: apply (bf16) -----------------------
            n_grp = nt // 4
            xqs = []
            for grp in range(n_grp):
                xq = xp.tile([128, 4 * 768], bf16, tag="xq")
                src = xsep_d[grp * 512:(grp + 1) * 512, :].rearrange(
                    "(g p) c -> p g c", p=128)
                nc.sync.dma_start(xq[:].rearrange("p (g c) -> p g c", c=768), src)
                xqs.append(xq)
            # apply (v7 structure): per 128-frame tile, ACT computes the
            # bias products P0, DVE the plain products P1/P2 (a quarter on
            # ACT for balance), then two 768-wide DVE adds; tile t's adds are
            # software-pipelined behind tile t+1's products so DVE never
            # reads a value it just wrote.
            pending = None

            def flush_pending():
                P0p, P1p, P2p, otp, obp, ggp = pending
                nc.vector.tensor_tensor(P0p[:], P0p[:], P1p[:], A_.add)
                nc.vector.tensor_tensor(otp[:, obp:obp + 768],
                                        P0p[:], P2p[:], A_.add)
                if ggp % 2 == 1:
                    dst = out_d[(ggp - 1) * 128:(ggp + 1) * 128, :].rearrange(
                        "(g p) c -> p g c", p=128)
                    nc.sync.dma_start(dst, otp[:].rearrange(
                        "p (g c) -> p g c", c=768))

            for grp in range(n_grp):
                xq = xqs[grp]
                for t in range(4):
                    gg = grp * 4 + t
                    base = t * 768
                    if t % 2 == 0:
                        ot = opool.tile([128, 2 * 768], bf16, tag="ot")
                    obase = (t % 2) * 768
                    P0 = p2p.tile([128, 768], bf16, tag="P0")
                    P1 = p2p.tile([128, 768], bf16, tag="P1")
                    P2 = p2p.tile([128, 768], bf16, tag="P2")
                    for bi in range(3):
                        rcol0 = R[:, bi * nt + gg: bi * nt + gg + 1]
                        rcol1 = R[:, (3 + bi) * nt + gg: (3 + bi) * nt + gg + 1]
                        rcol2 = R[:, (6 + bi) * nt + gg: (6 + bi) * nt + gg + 1]
                        tncol = R[:, (9 + bi) * nt + gg: (9 + bi) * nt + gg + 1]
                        x0 = xq[:, base:base + 256]
                        x1 = xq[:, base + 256:base + 512]
                        x2 = xq[:, base + 512:base + 768]
                        nc.scalar.activation(P0[:, bi * 256:(bi + 1) * 256],
                                             x0, AF.Identity,
                                             bias=tncol, scale=rcol0)
                        if (gg + bi) % 4 == 0:
                            nc.scalar.activation(
                                P1[:, bi * 256:(bi + 1) * 256], x1,
                                AF.Copy, scale=rcol1)
                        else:
                            nc.vector.tensor_scalar(
                                P1[:, bi * 256:(bi + 1) * 256],
                                x1, rcol1, None, A_.mult)
                        nc.vector.tensor_scalar(P2[:, bi * 256:(bi + 1) * 256],
                                                x2, rcol2, None, A_.mult)
                    if pending is not None:
                        flush_pending()
                    pending = (P0, P1, P2, ot, obase, gg)
            flush_pending()
            psc2.release()
            psop.release()

    if split_waits:
        _split_multiwait(nc)
    return nc


# ----------------------------------------------------------------------------
# Host-side preparation
# ----------------------------------------------------------------------------

def _prep_inputs(x, ref_x, align_idx):
    import ml_dtypes
    BF16 = ml_dtypes.bfloat16
    x = np.asarray(x, dtype=F32)
    ref_x = np.asarray(ref_x)
    idx = np.asarray(align_idx).astype(np.int64)
    L = x.shape[0]

    ref64 = ref_x.astype(np.float64)
    ref_c = (ref64 - ref64.mean(0)).astype(F32)        # [64, 3]

    xg = x[:, idx, :]                                   # [L, 64, 3]
    xgt = np.ascontiguousarray(xg.reshape(L, 192).T)    # f32 [192, L]

    xsep = np.ascontiguousarray(
        x.transpose(0, 2, 1)).reshape(L, 768).astype(BF16)

    W = np.zeros((192, 12), dtype=F32)
    for a in range(3):
        rows = 3 * np.arange(N_ALIGN) + a
        for b in range(3):
            W[rows, 3 * a + b] = ref_c[:, b]
        W[rows, 9 + a] = F32(1.0 / N_ALIGN)
    return xgt, xsep, W


# ----------------------------------------------------------------------------
# Runner: jit once, reuse
# ----------------------------------------------------------------------------

class _Runner:
    def __init__(self):
        import jax

        self.jax = jax
        self.nc = _build_program(LS)
        self._build_exec()

    def _build_exec(self):
        import jax
        from jax.sharding import Mesh, PartitionSpec
        from jax.experimental.shard_map import shard_map
        from concourse import mybir
        from concourse.bass2jax import (_bass_exec_p, install_neuronx_cc_hook,
                                        partition_id_tensor)

        install_neuronx_cc_hook()
        # surface compile-hook exceptions (PJRT swallows them)
        try:
            import libneuronxla
            import traceback
            if not getattr(libneuronxla, "_ant_logged_cc", False):
                _orig_cc = libneuronxla.neuronx_cc

                def _logged_cc(*a, **k):
                    try:
                        return _orig_cc(*a, **k)
                    except BaseException:
                        traceback.print_exc()
                        raise

                libneuronxla.neuronx_cc = _logged_cc
                libneuronxla._ant_logged_cc = True
        except ImportError:
            pass
        nc = self.nc

        part_name = (nc.partition_id_tensor.name
                     if nc.partition_id_tensor else None)
        in_names, out_names, out_avals = [], [], []
        for alloc in nc.m.functions[0].allocations:
            if not isinstance(alloc, mybir.MemoryLocationSet):
                continue
            name = alloc.memorylocations[0].name
            if alloc.kind == "ExternalInput":
                if name != part_name:
                    in_names.append(name)
            elif alloc.kind == "ExternalOutput":
                shape = tuple(alloc.tensor_shape)
                dtype = mybir.dt.np(alloc.dtype)
                out_names.append(name)
                out_avals.append(jax.core.ShapedArray(shape, dtype))
        self.in_names = list(in_names)
        self.out_names = list(out_names)
        n_params = len(in_names)
        all_names = in_names + out_names
        if part_name is not None:
            all_names = all_names + [part_name]

        def _body(*args):
            operands = list(args)
            if part_name is not None:
                operands.append(partition_id_tensor())
            outs = _bass_exec_p.bind(
                *operands,
                out_avals=tuple(out_avals),
                in_names=tuple(all_names),
                out_names=tuple(out_names),
                lowering_input_output_aliases=(),
                sim_require_finite=True,
                sim_require_nnan=True,
                nc=nc,
            )
            return tuple(outs)

        devices = jax.devices()[:N_CORES]
        mesh = Mesh(np.asarray(devices), ("core",))
        n_outs = len(out_names)
        in_specs = (PartitionSpec("core"),) * (n_params + n_outs)
        out_specs = (PartitionSpec("core"),) * n_outs
        self._fn = jax.jit(
            shard_map(_body, mesh=mesh, in_specs=in_specs,
                      out_specs=out_specs, check_rep=False),
            keep_unused=True,
        )
        self._zeros = [
            np.zeros((N_CORES * av.shape[0], *av.shape[1:]), av.dtype)
            for av in out_avals
        ]

    def stage(self, x, ref_x, align_idx):
        import ml_dtypes
        xgt, xsep, W = _prep_inputs(x, ref_x, align_idx)
        per_name = {
            "xgt": np.concatenate(
                [xgt[:, c * LS:(c + 1) * LS] for c in range(N_CORES)], axis=0),
            "xsep": xsep,
            "wm": np.concatenate([W] * N_CORES, axis=0),
            "ident": np.concatenate(
                [np.eye(12, dtype=F32)] * N_CORES, axis=0),
            "identb": np.concatenate(
                [np.eye(128).astype(ml_dtypes.bfloat16)] * N_CORES, axis=0),
        }
        args = [per_name[n] for n in self.in_names] + list(self._zeros)
        return [self.jax.device_put(a) for a in args]

    def run_staged(self, staged):
        return self._fn(*staged)

    def run(self, x, ref_x, align_idx):
        staged = self.stage(x, ref_x, align_idx)
        outs = self.run_staged(staged)
        out = np.asarray(outs[self.out_names.index("out")]).astype(np.float32)
        L = out.shape[0]
        return np.ascontiguousarray(
            out.reshape(L, 3, N_INP).transpose(0, 2, 1))


def _get_runner():
    global _RUNNER
    if _RUNNER is None:
        _RUNNER = _Runner()
    return _RUNNER


def kernel(x, ref_x, align_idx):
    runner = _get_runner()
    return runner.run(x, ref_x, align_idx).astype(np.float32)


if __name__ == "__main__":
    nc = _build_program(LS)
    print("built ok")


# revision 47
# speedup vs baseline: 1.2757x; 1.0034x over previous
"""Trainium2 Bass kernel for nn_AlignmentLayer (Kabsch alignment of L frames).

Strategy (pure data parallel over 8 NeuronCores, L/8 = 8192 frames per core):

Host-side (numpy, cheap layout work only):
  - ref_c = ref_x - mean(ref_x); gather xg = x[:, align_idx, :]  (align_idx is
    a host-known constant input, so the gather folds into data layout).
  - xgt: gathered atoms pre-transposed to [192, L] f32 so phase 1 needs
    zero on-chip transposes.
  - xsep: x in component-major layout [L, 3, 256] bf16 so phase-3 tensor ops
    are contiguous; output produced component-major bf16 and unpacked on host.
  - W: [192, 12] f32 weights mapping gathered rows to the 9 entries of
    A = xg^T @ ref_c and the 3 entries of the centroid x_c.

Device (per core), three phases:
  1. PE matmuls, weight-stationary: ET[12, ls] = W^T @ xgt in 512-frame
     strips (f32 for exact E — bf16 E perturbs near-singular frames), then
     PE-transposed back to E[128, nt*12] via identity matmuls.
  2. Math (DVE + Pool + ACT, batched [128, 64] ops): SVD-free Kabsch
     rotation. S = A^T A; lambda1 via trigonometric cubic (arctan+sin);
     v1 = best cross product of rows of (S - lambda1 I); (v2, v3) from a
     deflated 2x2 eigenproblem in the Householder complement of v1;
     u_i = normalize(A v_i); u3 = u1 x u2; R = sum u_i v_i^T; tneg = -x_c R.
     rsqrt/recip computed as Exp(-c*Ln(x)) on ACT; a greedy list scheduler
     splits the op DAG across DVE and Pool.
  3. Apply (bf16): per 128-frame tile and component b, products
     P0 = x0*R0b + tneg_b (ACT), P1 = x1*R1b, P2 = x2*R2b (DVE ts), then
     two 768-wide DVE adds produce the output tile.
"""

import numpy as np

L_FULL = 65536
N_INP = 256
N_ALIGN = 64
N_CORES = 8
LS = L_FULL // N_CORES          # frames per core
NT = LS // 128                  # 128-frame tiles per core (64)
F32 = np.float32

_RUNNER = None


# ----------------------------------------------------------------------------
# Math IR: record ops on virtual registers; a greedy list scheduler assigns
# each op to DVE ("V") or Pool ("G") (ACT ops pinned to "S"), then emission
# uses per-engine linear-scan slot allocation into one scratch tensor.
# ----------------------------------------------------------------------------

class _VR(int):
    """Virtual register id."""


# measured per-op engine costs at [128, 64] f32 (ns)
_COST = {
    ("tt", "V"): 150, ("tt", "G"): 300,
    ("ts", "V"): 115, ("ts", "G"): 260,
}
_ACT_COST = {"Ln": 240, "Exp": 350}
_XENG_NS = 250        # cross-engine result handoff penalty
_V_BIAS = 1.0         # apply follows math serially, so just balance math wall
_REBAL_NS = 800       # affinity hysteresis: rebalance only past this drift


class _MathIR:
    def __init__(self, alu):
        self.A_ = alu
        self.ops = []           # (kind, out, ins, extra)
        self.n = 0

    def _rec(self, kind, ins, extra=None, out=None):
        if out is None:
            out = _VR(self.n)
            self.n += 1
        self.ops.append((kind, out, list(ins), extra))
        return out

    def tt(self, op, a, b, out=None):
        return self._rec("tt", [a, b], op, out)

    def mul(self, a, b, out=None):
        return self.tt(self.A_.mult, a, b, out)

    def add(self, a, b, out=None):
        return self.tt(self.A_.add, a, b, out)

    def sub(self, a, b, out=None):
        return self.tt(self.A_.subtract, a, b, out)

    def ts(self, a, s1, op0, s2=None, op1=None, out=None):
        return self._rec("ts", [a], (float(s1), op0,
                                     None if s2 is None else float(s2), op1), out)

    def act(self, fn, a, scale=1.0, bias=None, out=None):
        return self._rec("act", [a], (fn, scale, bias), out)

    def rsqrt(self, nval):
        """1/sqrt(n) = Exp(-0.5*Ln(n)) on ACT (n must be > 0)."""
        from concourse import mybir
        AF = mybir.ActivationFunctionType
        ln = self.act(AF.Ln, nval)
        return self.act(AF.Exp, ln, scale=-0.5)

    def recip(self, nval):
        """1/n = Exp(-Ln(n)) on ACT (n must be > 0)."""
        from concourse import mybir
        AF = mybir.ActivationFunctionType
        ln = self.act(AF.Ln, nval)
        return self.act(AF.Exp, ln, scale=-1.0)

    def dot3(self, ax, ay, az, bx, by, bz):
        t1 = self.mul(ax, bx)
        t2 = self.mul(ay, by)
        s = self.add(t1, t2)
        t3 = self.mul(az, bz)
        return self.add(s, t3)

    def cross3(self, a, b):
        cx = self.sub(self.mul(a[1], b[2]), self.mul(a[2], b[1]))
        cy = self.sub(self.mul(a[2], b[0]), self.mul(a[0], b[2]))
        cz = self.sub(self.mul(a[0], b[1]), self.mul(a[1], b[0]))
        return [cx, cy, cz]

    def blend3(self, m, a, b):
        out = []
        for i in range(3):
            d = self.sub(a[i], b[i])
            out.append(self.add(b[i], self.mul(m, d)))
        return out


_RAW_LAT = 100        # same-engine RAW result latency (SBUF write ack)


def _schedule_math(ir):
    """Latency-aware list scheduling: all tensor ops on V, acts on S, and the
    EMISSION ORDER is chosen so dependent ops are spaced apart (back-to-back
    RAW chains pay the DVE write-ack latency). Returns (order, assign, clock).
    """
    n = len(ir.ops)
    eng, cost = [], []
    for kind, out, ins, extra in ir.ops:
        if kind == "act":
            fname = getattr(extra[0], "name", str(extra[0]))
            eng.append("S")
            cost.append(_ACT_COST.get(fname, 440))
        else:
            eng.append("V")
            cost.append(_COST[(kind, "V")])

    # dependency edges via vregs
    producer = {}
    deps = [[] for _ in range(n)]
    users = [[] for _ in range(n)]
    for i, (kind, out, ins, extra) in enumerate(ir.ops):
        for v in ins:
            if isinstance(v, _VR) and int(v) in producer:
                p = producer[int(v)]
                deps[i].append(p)
                users[p].append(i)
        if isinstance(out, _VR):
            producer[int(out)] = i

    # height = critical-path length to any sink
    height = [0] * n
    for i in range(n - 1, -1, -1):
        h = cost[i]
        for u in users[i]:
            h = max(h, cost[i] + height[u])
        height[i] = h

    indeg = [len(set(deps[i])) for i in range(n)]
    ready = [i for i in range(n) if indeg[i] == 0]
    clock = {"V": 0.0, "S": 0.0, "G": 0.0}
    fin = [0.0] * n
    done_deps = [set() for _ in range(n)]
    order = []
    import heapq
    while ready:
        # earliest feasible start per candidate
        best, best_key = None, None
        for i in ready:
            e = eng[i]
            est = clock[e]
            for p in set(deps[i]):
                lat = _RAW_LAT if eng[p] == e else _XENG_NS
                est = max(est, fin[p] + lat)
            stall = est - clock[e]
            key = (stall, -height[i])
            if best_key is None or key < best_key:
                best, best_key, best_est = i, key, est
        i = best
        ready.remove(i)
        e = eng[i]
        fin[i] = best_est + cost[i]
        clock[e] = fin[i]
        order.append(i)
        for u in users[i]:
            done_deps[u].add(i)
            if len(done_deps[u]) == len(set(deps[u])) and u not in ready \
                    and u not in order:
                ready.append(u)
    assert len(order) == n
    return order, eng, clock


def _emit_math(nc, ir, ms_ap, C, n_slots):
    """Emit recorded IR in the latency-aware schedule order. Vreg v lives in
    ms_ap[:, slot*C:(slot+1)*C]; slots partitioned per engine so WAR reuse
    stays engine-local."""
    order, assign, clock = _schedule_math(ir)

    # last use position in the EMISSION order
    pos = {op_i: k for k, op_i in enumerate(order)}
    last_use = {}
    for i, (kind, out, ins, extra) in enumerate(ir.ops):
        for v in ins:
            if isinstance(v, _VR):
                last_use[int(v)] = max(last_use.get(int(v), -1), pos[i])

    # per-engine slot ranges sized from peak live-value demand (emission order)
    peak = {"V": 0, "G": 0, "S": 0}
    live = {"V": 0, "G": 0, "S": 0}
    ends = {}
    for k, op_i in enumerate(order):
        kind, out, ins, extra = ir.ops[op_i]
        e = assign[op_i]
        if isinstance(out, _VR):
            live[e] += 1
            peak[e] = max(peak[e], live[e])
            ends[int(out)] = e
        for vi in {int(v) for v in ins if isinstance(v, _VR)}:
            if last_use.get(vi) == k and vi in ends:
                live[ends[vi]] -= 1
    need = {e: peak[e] + 1 for e in peak}
    assert sum(need.values()) <= n_slots, f"need {need} > {n_slots} slots"
    ranges, lo = {}, 0
    for e in ("V", "G", "S"):
        ranges[e] = (lo, lo + need[e])
        lo += need[e]
    free = {e: list(range(r[1] - 1, r[0] - 1, -1)) for e, r in ranges.items()}
    slot_of = {}
    eng_of_slot = {}

    def ap_of(v):
        if isinstance(v, _VR):
            s = slot_of[int(v)]
            return ms_ap[:, s * C:(s + 1) * C]
        return v  # external AP

    for k, op_i in enumerate(order):
        kind, out, ins, extra = ir.ops[op_i]
        e = assign[op_i]
        if isinstance(out, _VR):
            assert free[e], f"scratch slots exhausted for engine {e}"
            slot = free[e].pop()
            slot_of[int(out)] = slot
            eng_of_slot[slot] = e
            out_ap = ms_ap[:, slot * C:(slot + 1) * C]
        else:
            out_ap = out
        in_aps = [ap_of(v) for v in ins]
        eng = {"V": nc.vector, "G": nc.gpsimd, "S": nc.scalar}[e]
        if kind == "tt":
            eng.tensor_tensor(out_ap, in_aps[0], in_aps[1], extra)
        elif kind == "ts":
            s1, op0, s2, op1 = extra
            if s2 is None:
                eng.tensor_scalar(out_ap, in_aps[0], s1, None, op0)
            else:
                eng.tensor_scalar(out_ap, in_aps[0], s1, s2, op0, op1)
        elif kind == "act":
            fn, scale, bias = extra
            if bias is None:
                nc.scalar.activation(out_ap, in_aps[0], fn, scale=scale)
            else:
                nc.scalar.activation(out_ap, in_aps[0], fn, scale=scale,
                                     bias=bias)
        else:
            raise ValueError(kind)
        for vi in {int(v) for v in ins if isinstance(v, _VR)}:
            if last_use.get(vi) == k:
                s = slot_of[vi]
                free[eng_of_slot[s]].append(s)
    return clock


def _record_math(ir, Ev, Rv, consts):
    """Record the whole rotation math on the IR. Ev/Rv are [128, 12, C] views
    (strided entry slices); consts maps name -> [128,1] const AP."""
    from concourse import mybir
    AF = mybir.ActivationFunctionType
    A_ = ir.A_

    Ae = [[Ev[:, 3 * a + b, :] for b in range(3)] for a in range(3)]
    me = [Ev[:, 9 + a, :] for a in range(3)]

    # S = A^T A (6 unique entries)
    Smat = {}
    for bi in range(3):
        for ci in range(bi, 3):
            Smat[(bi, ci)] = ir.dot3(Ae[0][bi], Ae[1][bi], Ae[2][bi],
                                     Ae[0][ci], Ae[1][ci], Ae[2][ci])

    def S(i, j):
        return Smat[(min(i, j), max(i, j))]

    q = ir.ts(ir.add(ir.add(S(0, 0), S(1, 1)), S(2, 2)), 1.0 / 3.0, A_.mult)
    P00 = ir.sub(S(0, 0), q)
    P11 = ir.sub(S(1, 1), q)
    P22 = ir.sub(S(2, 2), q)
    sq01 = ir.mul(S(0, 1), S(0, 1))
    sq02 = ir.mul(S(0, 2), S(0, 2))
    sq12 = ir.mul(S(1, 2), S(1, 2))
    diagsq = ir.add(ir.add(ir.mul(P00, P00), ir.mul(P11, P11)), ir.mul(P22, P22))
    offsq = ir.add(ir.add(sq01, sq02), sq12)
    p2v = ir.add(diagsq, ir.ts(offsq, 2.0, A_.mult))
    p2c = ir.ts(ir.ts(p2v, 1.0 / 6.0, A_.mult), 1e-30, A_.max)
    ln_p = ir.act(AF.Ln, p2c)
    pval = ir.act(AF.Exp, ln_p, scale=0.5)       # sqrt(p2c)
    pinv3 = ir.act(AF.Exp, ln_p, scale=-1.5)     # p2c^-1.5

    c0 = ir.sub(ir.mul(P11, P22), sq12)
    c1c = ir.sub(ir.mul(S(0, 1), P22), ir.mul(S(1, 2), S(0, 2)))
    c2c = ir.sub(ir.mul(S(0, 1), S(1, 2)), ir.mul(P11, S(0, 2)))
    detB = ir.add(ir.sub(ir.mul(P00, c0), ir.mul(S(0, 1), c1c)),
                  ir.mul(S(0, 2), c2c))
    rr = ir.ts(ir.mul(detB, pinv3), 0.5, A_.mult, 0.9999995, A_.min)
    rr = ir.ts(rr, -0.9999995, A_.max)

    omr = ir.ts(ir.mul(rr, rr), -1.0, A_.mult, 1.0, A_.add)
    rs = ir.rsqrt(omr)
    uu = ir.mul(rr, rs)
    # arctan(u) with range reduction — ACT Arctan domain is [-pi/2, pi/2]:
    # |u|<=1: a = arctan(|u|); |u|>1: pi/2 - arctan(1/|u|); then apply sign.
    au = ir.tt(A_.max, uu, ir.ts(uu, -1.0, A_.mult))      # |u|
    inv = ir.recip(ir.ts(au, 1e-30, A_.max))
    z = ir.tt(A_.min, au, inv)
    az = ir.act(AF.Arctan, z)
    dz = ir.ts(az, -1.0, A_.mult, float(np.pi / 2), A_.add)
    mge = ir.ts(au, 1.0, A_.is_ge)                        # |u| >= 1
    mle = ir.act(AF.Identity, mge, scale=-1.0, bias=consts["one"])  # 1 - that
    res_abs = ir.add(dz, ir.mul(mle, ir.sub(az, dz)))
    sgn_u = ir.ts(ir.ts(uu, 0.0, A_.is_ge), 2.0, A_.mult, -1.0, A_.add)
    at = ir.mul(res_abs, sgn_u)
    c1t = ir.act(AF.Sin, at, scale=1.0 / 3.0, bias=consts["pi3"])
    lam1 = ir.add(q, ir.ts(ir.mul(pval, c1t), 2.0, A_.mult))

    # v1 = best cross of rows of (S - lam1 I)
    D0 = ir.sub(S(0, 0), lam1)
    D1 = ir.sub(S(1, 1), lam1)
    D2 = ir.sub(S(2, 2), lam1)
    rows = [
        [D0, S(0, 1), S(0, 2)],
        [S(0, 1), D1, S(1, 2)],
        [S(0, 2), S(1, 2), D2],
    ]
    best, bn = None, None
    for (i, j) in [(0, 1), (0, 2)]:
        c = ir.cross3(rows[i], rows[j])
        n = ir.dot3(c[0], c[1], c[2], c[0], c[1], c[2])
        if best is None:
            best, bn = c, n
        else:
            m = ir.tt(A_.is_gt, n, bn)
            best = ir.blend3(m, c, best)
            bn = ir.add(bn, ir.mul(m, ir.sub(n, bn)))
    inv1 = ir.rsqrt(ir.ts(bn, 1e-37, A_.max))
    v1 = [ir.mul(best[0], inv1), ir.mul(best[1], inv1), ir.mul(best[2], inv1)]

    # (w2, w3): orthonormal complement of v1 via Householder columns.
    # H = I - h h^T/(1+a), h = v1 + s*e0, s = sign(v1x), a = s*v1x = |v1x|.
    sgn = ir.ts(ir.ts(v1[0], 0.0, A_.is_ge), 2.0, A_.mult, -1.0, A_.add)
    alpha = ir.mul(sgn, v1[0])
    denom = ir.ts(alpha, 1.0, A_.add)                     # 1 + |v1x| in [1,2]
    rden = ir.recip(denom)
    h0 = ir.add(v1[0], sgn)
    hyr = ir.mul(v1[1], rden)
    nhyr = ir.ts(hyr, -1.0, A_.mult)
    w2 = [ir.mul(h0, nhyr),
          ir.ts(ir.mul(v1[1], hyr), -1.0, A_.mult, 1.0, A_.add),
          ir.mul(v1[2], nhyr)]
    w3 = ir.cross3(v1, w2)

    # deflated 2x2 eigenproblem in span{w2, w3}; c2x via trace identity.
    Sw2 = [ir.dot3(S(bi, 0), S(bi, 1), S(bi, 2), w2[0], w2[1], w2[2])
           for bi in range(3)]
    a2x = ir.dot3(w2[0], w2[1], w2[2], Sw2[0], Sw2[1], Sw2[2])
    b2x = ir.dot3(Sw2[0], Sw2[1], Sw2[2], w3[0], w3[1], w3[2])
    trq = ir.act(AF.Identity, q, scale=3.0)
    c2x = ir.sub(trq, ir.add(lam1, a2x))

    half = ir.ts(ir.sub(a2x, c2x), 0.5, A_.mult)
    mpos = ir.ts(half, 0.0, A_.is_ge)
    sgn2 = ir.ts(mpos, 2.0, A_.mult, -1.0, A_.add)
    habs = ir.mul(sgn2, half)
    rad2 = ir.ts(ir.add(ir.mul(half, half), ir.mul(b2x, b2x)), 1e-37, A_.max)
    rad = ir.act(AF.Sqrt, rad2)
    pos = ir.ts(ir.add(habs, rad), 1e-37, A_.max)
    tq = ir.mul(ir.mul(b2x, ir.recip(pos)), sgn2)
    c2i = ir.rsqrt(ir.ts(ir.mul(tq, tq), 1.0, A_.add))
    s2i = ir.mul(tq, c2i)
    tb = ir.mul(tq, b2x)
    lamA = ir.add(a2x, tb)
    lamB = ir.sub(c2x, tb)
    mAB = ir.tt(A_.is_ge, lamA, lamB)
    vA = [ir.add(ir.mul(c2i, w2[i]), ir.mul(s2i, w3[i])) for i in range(3)]
    vB = [ir.sub(ir.mul(c2i, w3[i]), ir.mul(s2i, w2[i])) for i in range(3)]
    v2 = ir.blend3(mAB, vA, vB)
    v3 = ir.cross3(v1, v2)

    def Avec(v):
        return [ir.dot3(Ae[ai][0], Ae[ai][1], Ae[ai][2], v[0], v[1], v[2])
                for ai in range(3)]

    b1 = Avec(v1)
    n1 = ir.dot3(b1[0], b1[1], b1[2], b1[0], b1[1], b1[2])
    i1 = ir.rsqrt(ir.ts(n1, 1e-37, A_.max))
    u1 = [ir.mul(b1[i], i1) for i in range(3)]

    b2v = Avec(v2)
    dd = ir.dot3(u1[0], u1[1], u1[2], b2v[0], b2v[1], b2v[2])
    b2o = [ir.sub(b2v[i], ir.mul(dd, u1[i])) for i in range(3)]
    n2 = ir.dot3(b2o[0], b2o[1], b2o[2], b2o[0], b2o[1], b2o[2])
    i2 = ir.rsqrt(ir.ts(n2, 1e-37, A_.max))
    u2 = [ir.mul(b2o[i], i2) for i in range(3)]

    u3 = ir.cross3(u1, u2)

    us = [u1, u2, u3]
    vs = [v1, v2, v3]
    Re = [[None] * 3 for _ in range(3)]
    for ai in range(3):
        for bi in range(3):
            t1 = ir.mul(us[0][ai], vs[0][bi])
            t2 = ir.mul(us[1][ai], vs[1][bi])
            sgm = ir.add(t1, t2)
            t3 = ir.mul(us[2][ai], vs[2][bi])
            r = ir.add(sgm, t3)
            Re[ai][bi] = r
            ir.act(AF.Identity, r, out=Rv[:, 3 * ai + bi, :])

    mn = [ir.act(AF.Identity, me[i], scale=-1.0) for i in range(3)]
    for bi in range(3):
        t1 = ir.mul(mn[0], Re[0][bi])
        t2 = ir.mul(mn[1], Re[1][bi])
        sgm = ir.add(t1, t2)
        t3 = ir.mul(mn[2], Re[2][bi])
        ir.add(sgm, t3, out=Rv[:, 9 + bi, :])


# ----------------------------------------------------------------------------
# Bass program
# ----------------------------------------------------------------------------

def _split_multiwait(nc):
    """This walrus build encodes at most ONE semaphore wait per instruction,
    but Tile emits several. Split extras into standalone EventSemaphore
    (pure wait) instructions on the same engine, immediately before."""
    from concourse import mybir
    import bass_rust

    n_split = 0
    for fn in nc.m.functions:
        for blk in fn.blocks:
            new = []
            for ins in blk.instructions:
                si = ins.sync_info
                if si is not None and si.on_wait is not None and len(si.on_wait) > 1:
                    waits = list(si.on_wait)
                    for k, w in enumerate(waits[:-1]):
                        new.append(mybir.InstEventSemaphore(
                            name=f"{ins.name}-w{k}",
                            engine=ins.engine,
                            sync_info=bass_rust.SyncInfo(
                                on_wait=[w], on_update=[]),
                        ))
                        n_split += 1
                    ins.sync_info = bass_rust.SyncInfo(
                        on_wait=[waits[-1]],
                        on_update=list(si.on_update or []))
                new.append(ins)
            blk.instructions = new
    return n_split


def _build_program(ls=LS, n_slots=62, split_waits=True, prefetch=6):
    import concourse.bass as bass
    import concourse.tile as tile
    from concourse import mybir

    f32 = mybir.dt.float32
    bf16 = mybir.dt.bfloat16
    A_ = mybir.AluOpType
    AF = mybir.ActivationFunctionType

    nt = ls // 128
    C = nt

    nc = bass.Bass("TRN2", target_bir_lowering=False, debug=False)

    f32r = mybir.dt.float32r
    xgt_d = nc.dram_tensor("xgt", [192, ls], f32r, kind="ExternalInput").ap()
    xsep_d = nc.dram_tensor("xsep", [ls, 768], bf16, kind="ExternalInput").ap()
    w_d = nc.dram_tensor("wm", [192, 12], f32r, kind="ExternalInput").ap()
    id_d = nc.dram_tensor("ident", [12, 12], f32, kind="ExternalInput").ap()
    idb_d = nc.dram_tensor("identb", [128, 128], bf16, kind="ExternalInput").ap()
    out_d = nc.dram_tensor("out", [ls, 768], bf16, kind="ExternalOutput").ap()

    with tile.TileContext(nc) as tc:
        with (
            tc.tile_pool(name="wp", bufs=1) as wp,
            tc.tile_pool(name="gp_", bufs=1) as gpool,
            tc.tile_pool(name="ep", bufs=1) as ep,
            tc.tile_pool(name="xp", bufs=prefetch) as xp,
            tc.tile_pool(name="p2", bufs=6) as p2p,
            tc.tile_pool(name="op_", bufs=3) as opool,
        ):
            # ---------------- constants / weights ----------------
            identb = wp.tile([128, 128], bf16, tag="identb")
            nc.sync.dma_start(identb[:], idb_d)
            w0 = wp.tile([128, 12], f32r, tag="w0")
            w1 = wp.tile([64, 12], f32r, tag="w1")
            nc.sync.dma_start(w0[:], w_d[0:128, :])
            nc.sync.dma_start(w1[:], w_d[128:192, :])
            ident = wp.tile([12, 12], f32, tag="ident")
            nc.sync.dma_start(ident[:], id_d)

            ET = ep.tile([12, ls], f32, tag="ET")
            # E and R are ENTRY-MAJOR [128, e*nt + g]: math operands become
            # contiguous 64-column slices (DVE fast path, Pool software loop).
            E = ep.tile([128, nt * 12], f32, tag="E")
            R = ep.tile([128, nt * 12], f32, tag="R")
            MS = ep.tile([128, n_slots * C], f32, tag="MS")
            cst = ep.tile([128, 4], f32, tag="cst")
            nc.gpsimd.memset(cst[:, 0:1], float(np.pi / 3))
            nc.gpsimd.memset(cst[:, 1:2], float(np.pi / 2))
            nc.gpsimd.memset(cst[:, 2:3], 1.0)
            nc.gpsimd.memset(cst[:, 3:4], -1.0)
            consts = {"pi3": cst[:, 0:1], "pi2": cst[:, 1:2],
                      "one": cst[:, 2:3], "neg1": cst[:, 3:4]}
            Ev = E[:].rearrange("p (e g) -> p e g", e=12)
            Rv = R[:].rearrange("p (e g) -> p e g", e=12)
            Eg = E[:].rearrange("p (e g) -> p g e", e=12)

            # ---------------- phase 1: ET = W^T @ xgt (f32r), then PE ------
            # transposes back to E. Dummy PE matmuls absorb each DMA's
            # semaphore into the PE's observed clock (Matmult ISA slot holds
            # at most ONE wait). Phase-1 PSUM pools are scoped so their banks
            # free up for the apply-phase accumulator pool.
            psp = tc.alloc_tile_pool(name="ps", bufs=2, space="PSUM")
            pstp = tc.alloc_tile_pool(name="pst", bufs=2, space="PSUM")
            pss = tc.alloc_tile_pool(name="ps2", bufs=1, space="PSUM")
            ps_scr = pss.tile([128, 12], f32, tag="scr")
            nc.tensor.matmul(ps_scr[0:12, 0:12], w0[:, 0:12], w0[:],
                             start=True, stop=True)
            nc.tensor.matmul(ps_scr[0:12, 0:12], w1[:, 0:12], w1[:],
                             start=True, stop=True)
            n_strip = ls // 1024
            slabs = []
            for s in range(n_strip):
                sl0 = gpool.tile([128, 1024], f32r, tag=f"g0_{s}")
                sl1 = gpool.tile([64, 1024], f32r, tag=f"g1_{s}")
                nc.sync.dma_start(sl0[:], xgt_d[0:128, s * 1024:(s + 1) * 1024])
                nc.sync.dma_start(sl1[:], xgt_d[128:192, s * 1024:(s + 1) * 1024])
                slabs.append((sl0, sl1))
            for s in range(n_strip):
                sl0, sl1 = slabs[s]
                nc.tensor.matmul(ps_scr[0:12, 0:12], sl0[:, 0:12], sl0[:, 0:12],
                                 start=True, stop=True)
                nc.tensor.matmul(ps_scr[0:12, 0:12], sl1[:, 0:12], sl1[:, 0:12],
                                 start=True, stop=True)
                for h in range(2):
                    psET = psp.tile([12, 512], f32, tag="psET")
                    nc.tensor.matmul(psET[:], w0[:],
                                     sl0[:, h * 512:(h + 1) * 512],
                                     start=True, stop=False)
                    nc.tensor.matmul(psET[:], w1[:],
                                     sl1[:, h * 512:(h + 1) * 512],
                                     start=False, stop=True)
                    nc.scalar.copy(
                        ET[:, s * 1024 + h * 512:s * 1024 + (h + 1) * 512],
                        psET[:])
                # transpose this strip's 8 frame-groups back to E
                psT = pstp.tile([128, 96], f32, tag="psT")
                for k in range(8):
                    g = s * 8 + k
                    nc.tensor.transpose(psT[:, k * 12:(k + 1) * 12],
                                        ET[:, g * 128:(g + 1) * 128],
                                        ident[:])
                nc.scalar.copy(
                    Eg[:, s * 8:(s + 1) * 8, :],
                    psT[:].rearrange("p (g e) -> p g e", e=12))
            pss.release()
            pstp.release()
            psp.release()
            psop = tc.alloc_tile_pool(name="pso", bufs=2, space="PSUM")
            psc2 = tc.alloc_tile_pool(name="psc2", bufs=1, space="PSUM")
            scr2 = psc2.tile([16, 12], mybir.dt.float32, tag="scr2")

            # ---------------- phase 2: rotation math ----------------------
            ir = _MathIR(A_)
            _record_math(ir, Ev, Rv, consts)
            _emit_math(nc, ir, MS[:], C, n_slots)

            # ---------------- phase 3: apply (bf16) -----------------------
            n_grp = nt // 4
            xqs = []
            for grp in range(n_grp):
                xq = xp.tile([128, 4 * 768], bf16, tag="xq")
                src = xsep_d[grp * 512:(grp + 1) * 512, :].rearrange(
                    "(g p) c -> p g c", p=128)
                nc.sync.dma_start(xq[:].rearrange("p (g c) -> p g c", c=768), src)
                xqs.append(xq)
            # apply (v7 structure): per 128-frame tile, ACT computes the
            # bias products P0, DVE the plain products P1/P2 (a quarter on
            # ACT for balance), then two 768-wide DVE adds; tile t's adds are
            # software-pipelined behind tile t+1's products so DVE never
            # reads a value it just wrote.
            pending = None

            def flush_pending():
                P0p, P1p, P2p, otp, obp, ggp = pending
                nc.vector.tensor_tensor(P0p[:], P0p[:], P1p[:], A_.add)
                nc.vector.tensor_tensor(otp[:, obp:obp + 768],
                                        P0p[:], P2p[:], A_.add)
                if ggp % 2 == 1:
                    dst = out_d[(ggp - 1) * 128:(ggp + 1) * 128, :].rearrange(
                        "(g p) c -> p g c", p=128)
                    nc.sync.dma_start(dst, otp[:].rearrange(
                        "p (g c) -> p g c", c=768))

            for grp in range(n_grp):
                xq = xqs[grp]
                for t in range(4):
                    gg = grp * 4 + t
                    base = t * 768
                    if t % 2 == 0:
                        ot = opool.tile([128, 2 * 768], bf16, tag="ot")
                    obase = (t % 2) * 768
                    P0 = p2p.tile([128, 768], bf16, tag="P0")
                    P1 = p2p.tile([128, 768], bf16, tag="P1")
                    P2 = p2p.tile([128, 768], bf16, tag="P2")
                    for bi in range(3):
                        rcol0 = R[:, bi * nt + gg: bi * nt + gg + 1]
                        rcol1 = R[:, (3 + bi) * nt + gg: (3 + bi) * nt + gg + 1]
                        rcol2 = R[:, (6 + bi) * nt + gg: (6 + bi) * nt + gg + 1]
                        tncol = R[:, (9 + bi) * nt + gg: (9 + bi) * nt + gg + 1]
                        x0 = xq[:, base:base + 256]
                        x1 = xq[:, base + 256:base + 512]
                        x2 = xq[:, base + 512:base + 768]
                        nc.scalar.activation(P0[:, bi * 256:(bi + 1) * 256],
                                             x0, AF.Identity,
                                             bias=tncol, scale=rcol0)
                        if (gg + bi) % 4 == 0:
                            nc.scalar.activation(
                                P1[:, bi * 256:(bi + 1) * 256], x1,
                                AF.Copy, scale=rcol1)
                        else:
                            nc.vector.tensor_scalar(
                                P1[:, bi * 256:(bi + 1) * 256],
                                x1, rcol1, None, A_.mult)
                        nc.vector.tensor_scalar(P2[:, bi * 256:(bi + 1) * 256],
                                                x2, rcol2, None, A_.mult)
                    if pending is not None:
                        flush_pending()
                    pending = (P0, P1, P2, ot, obase, gg)
            flush_pending()
            psc2.release()
            psop.release()

    if split_waits:
        _split_multiwait(nc)
    return nc


# ----------------------------------------------------------------------------
# Host-side preparation
# ----------------------------------------------------------------------------

def _prep_inputs(x, ref_x, align_idx):
    import ml_dtypes
    BF16 = ml_dtypes.bfloat16
    x = np.asarray(x, dtype=F32)
    ref_x = np.asarray(ref_x)
    idx = np.asarray(align_idx).astype(np.int64)
    L = x.shape[0]

    ref64 = ref_x.astype(np.float64)
    ref_c = (ref64 - ref64.mean(0)).astype(F32)        # [64, 3]

    xg = x[:, idx, :]                                   # [L, 64, 3]
    xgt = np.ascontiguousarray(xg.reshape(L, 192).T)    # f32 [192, L]

    xsep = np.ascontiguousarray(
        x.transpose(0, 2, 1)).reshape(L, 768).astype(BF16)

    W = np.zeros((192, 12), dtype=F32)
    for a in range(3):
        rows = 3 * np.arange(N_ALIGN) + a
        for b in range(3):
            W[rows, 3 * a + b] = ref_c[:, b]
        W[rows, 9 + a] = F32(1.0 / N_ALIGN)
    return xgt, xsep, W


# ----------------------------------------------------------------------------
# Runner: jit once, reuse
# ----------------------------------------------------------------------------

class _Runner:
    def __init__(self):
        import jax

        self.jax = jax
        self.nc = _build_program(LS)
        self._build_exec()

    def _build_exec(self):
        import jax
        from jax.sharding import Mesh, PartitionSpec
        from jax.experimental.shard_map import shard_map
        from concourse import mybir
        from concourse.bass2jax import (_bass_exec_p, install_neuronx_cc_hook,
                                        partition_id_tensor)

        install_neuronx_cc_hook()
        # surface compile-hook exceptions (PJRT swallows them)
        try:
            import libneuronxla
            import traceback
            if not getattr(libneuronxla, "_ant_logged_cc", False):
                _orig_cc = libneuronxla.neuronx_cc

                def _logged_cc(*a, **k):
                    try:
                        return _orig_cc(*a, **k)
                    except BaseException:
                        traceback.print_exc()
                        raise

                libneuronxla.neuronx_cc = _logged_cc
                libneuronxla._ant_logged_cc = True
        except ImportError:
            pass
        nc = self.nc

        part_name = (nc.partition_id_tensor.name
                     if nc.partition_id_tensor else None)
        in_names, out_names, out_avals = [], [], []
        for alloc in nc.m.functions[0].allocations:
            if not isinstance(alloc, mybir.MemoryLocationSet):
                continue
            name = alloc.memorylocations[0].name
            if alloc.kind == "ExternalInput":
                if name != part_name:
                    in_names.append(name)
            elif alloc.kind == "ExternalOutput":
                shape = tuple(alloc.tensor_shape)
                dtype = mybir.dt.np(alloc.dtype)
                out_names.append(name)
                out_avals.append(jax.core.ShapedArray(shape, dtype))
        self.in_names = list(in_names)
        self.out_names = list(out_names)
        n_params = len(in_names)
        all_names = in_names + out_names
        if part_name is not None:
            all_names = all_names + [part_name]

        def _body(*args):
            operands = list(args)
            if part_name is not None:
                operands.append(partition_id_tensor())
            outs = _bass_exec_p.bind(
                *operands,
                out_avals=tuple(out_avals),
                in_names=tuple(all_names),
                out_names=tuple(out_names),
                lowering_input_output_aliases=(),
                sim_require_finite=True,
                sim_require_nnan=True,
                nc=nc,
            )
            return tuple(outs)

        devices = jax.devices()[:N_CORES]
        mesh = Mesh(np.asarray(devices), ("core",))
        n_outs = len(out_names)
        in_specs = (PartitionSpec("core"),) * (n_params + n_outs)
        out_specs = (PartitionSpec("core"),) * n_outs
        self._fn = jax.jit(
            shard_map(_body, mesh=mesh, in_specs=in_specs,
                      out_specs=out_specs, check_rep=False),
            keep_unused=True,
        )
        self._zeros = [
            np.zeros((N_CORES * av.shape[0], *av.shape[1:]), av.dtype)
            for av in out_avals
        ]

    def stage(self, x, ref_x, align_idx):
        import ml_dtypes
        xgt, xsep, W = _prep_inputs(x, ref_x, align_idx)
        per_name = {
            "xgt": np.concatenate(
                [xgt[:, c * LS:(c + 1) * LS] for c in range(N_CORES)], axis=0),
            "xsep": xsep,
            "wm": np.concatenate([W] * N_CORES, axis=0),
            "ident": np.concatenate(
                [np.eye(12, dtype=F32)] * N_CORES, axis=0),
            "identb": np.concatenate(
                [np.eye(128).astype(ml_dtypes.bfloat16)] * N_CORES, axis=0),
        }
        args = [per_name[n] for n in self.in_names] + list(self._zeros)
        return [self.jax.device_put(a) for a in args]

    def run_staged(self, staged):
        return self._fn(*staged)

    def run(self, x, ref_x, align_idx):
        staged = self.stage(x, ref_x, align_idx)
        outs = self.run_staged(staged)
        out = np.asarray(outs[self.out_names.index("out")]).astype(np.float32)
        L = out.shape[0]
        return np.ascontiguousarray(
            out.reshape(L, 3, N_INP).transpose(0, 2, 1))


def _get_runner():
    global _RUNNER
    if _RUNNER is None:
        _RUNNER = _Runner()
    return _RUNNER


def kernel(x, ref_x, align_idx):
    runner = _get_runner()
    return runner.run(x, ref_x, align_idx).astype(np.float32)


if __name__ == "__main__":
    nc = _build_program(LS)
    print("built ok")
